# revision 1
# baseline (speedup 1.0000x reference)
"""Trainium2 Bass kernel for nn_Bihomogeneous_k3.

Math (per batch row, complex z of dim 5 given as z_re/z_im):
  zz[m]   = z_i z_j z_k for the 35 triples i<=j<=k (lexicographic)
  prod    = zz[p] * conj(zz[q]) for the 630 pairs p<=q (lexicographic)
  out     = [Re(prod) (630) | Im(prod) on strict pairs p<q (595)]   -> [B, 1225]

Distribution: pure data parallel over 8 NeuronCores (batch sharded).

Per-core design (B_local = 16384 rows):
  Layout: batch-major megatiles [128 partitions, G=32 groups, features],
  row b = mt*4096 + p*32 + g. All f32.
  - zz stage: complex mults via c-packed tensor_tensor ops with broadcast
    (step-0) and reversed (negative-step) access patterns; on DVE + GPSIMD.
  - pair products, per p-block (re: Rp*R[p:] + Ip*I[p:]; im strict:
    Ip*R[p+1:] - Rp*I[p+1:], via a negated-im copy of zz):
      * DVE writes one product half straight into PSUM,
      * the other half goes to SBUF (DVE or GPSIMD, greedy-balanced),
      * ONE identity-weight fp32 matmul per 512-elem piece accumulates it
        onto the PSUM half (start=False; exact for fp32; PSUM has_written
        bits are primed at kernel start so first-touch accumulates too),
      * ScalarE (ACT) drains PSUM -> SBUF output chunks; HWDGE DMAs out.
  - tiny blocks (w<=8) skip PSUM: mult + add on DVE/GPSIMD directly.
Cost-model estimate ~343us/core vs ~230us HBM-write roofline; engines are
balanced within ~15% of each other (DVE/GPS/PE/DMA/ACT).
"""
import sys

sys.path.insert(0, "/opt/trn_rl_repo")

import numpy as np

N = 5
NC = 8
B_FULL = 131072
B_LOCAL = B_FULL // NC
P = 128
G = 64
ROWS_PER_MT = P * G  # 4096
N_MT = B_LOCAL // ROWS_PER_MT  # 4

# ---- index tables (python-time constants) ----
TRIPLES = [(i, j, k) for i in range(N) for j in range(N) for k in range(N) if i <= j <= k]
M = len(TRIPLES)  # 35
WPAIRS = [(i, j) for i in range(N) for j in range(i, N)]  # 15, lex order
WOFF = {}
_o = 0
for (i, j) in WPAIRS:
    WOFF[(i, j)] = _o
    _o += 1
# zz offsets: triples are (i,j) pairs each followed by k=j..4 (lex order)
ZOFF = {}
_o = 0
for (i, j) in WPAIRS:
    ZOFF[(i, j)] = _o
    _o += N - j
assert _o == M

# re block p covers output cols [REOFF[p], REOFF[p]+35-p); im after 630
REOFF = np.concatenate([[0], np.cumsum([M - p for p in range(M)])]).astype(int)
IMOFF = np.concatenate([[0], np.cumsum([M - 1 - p for p in range(M - 1)])]).astype(int)
N_RE = int(REOFF[M])        # 630
N_IM = int(IMOFF[M - 1])    # 595
N_OUT = N_RE + N_IM         # 1225


# output column chunks (block-aligned). Each entry: (colbase, cols, blocks)
# where blocks is a list of ("re"/"im", p, block_col_base_in_chunk)
def _make_chunks(max_cols=448):
    blocks = []
    for p in range(M):
        blocks.append(("re", p, int(REOFF[p]), M - p))
    for p in range(M - 1):
        blocks.append(("im", p, N_RE + int(IMOFF[p]), M - 1 - p))
    chunks = []
    cur = []
    base = 0
    cols = 0
    for kind, p, cb, w in blocks:
        if cols + w > max_cols and cur:
            chunks.append((base, cols, cur))
            base = cb
            cols = 0
            cur = []
        cur.append((kind, p, cb - base, w))
        cols += w
    if cur:
        chunks.append((base, cols, cur))
    return chunks



def _ap(base_ap, offset_elems, dims, bassmod):
    """Build a raw AP from a tile's base AP: dims = [[step, count], ...] in
    elements, offset_elems added to the base offset."""
    return bassmod.AP(tensor=base_ap.tensor, offset=base_ap.offset + offset_elems,
                      ap=[list(base_ap.ap[0])] + [list(d) for d in dims])


def build_bass(n_mt=N_MT, g=G):
    import concourse.bacc as bacc
    import concourse.bass as bass
    import concourse.tile as tile
    from concourse import mybir
    from contextlib import ExitStack

    f32 = mybir.dt.float32
    b_local = P * g * n_mt

    nc = bacc.Bacc(None)
    z_re_d = nc.dram_tensor("z_re", [b_local, N], f32, kind="ExternalInput")
    z_im_d = nc.dram_tensor("z_im", [b_local, N], f32, kind="ExternalInput")
    ident_d = nc.dram_tensor("ident", [P, P], f32, kind="ExternalInput")
    out_d = nc.dram_tensor("out", [b_local, N_OUT], f32, kind="ExternalOutput")

    chunk_cols = 448 if g <= 32 else 160
    chunks = _make_chunks(chunk_cols)
    piece_w = max(4, 1024 // g)

    # greedy DVE/GPS load balancing (units: ns, cost-model calibrated)
    # DVE sbuf op ~ 121 + 1.042*fd ; DVE psum-dest op ~ 250 + 1.042*fd
    # GPS op ~ 95 + 1.984*fd (fp32 TT at 0.42 efficiency)
    eng_load = {"v": 0.0, "g": 0.0}

    def pick(fd):
        cv = eng_load["v"] + 121 + 1.042 * fd
        cg = eng_load["g"] + 95 + 1.984 * fd
        if cv <= cg:
            eng_load["v"] = cv
            return nc.vector
        eng_load["g"] = cg
        return nc.gpsimd

    with tile.TileContext(nc) as tc:
        with ExitStack() as ctx:
            const_pool = ctx.enter_context(tc.tile_pool(name="const", bufs=1))
            zpool = ctx.enter_context(tc.tile_pool(name="zp", bufs=2))
            wpool = ctx.enter_context(tc.tile_pool(name="wp", bufs=2))
            zzpool = ctx.enter_context(tc.tile_pool(name="zzp", bufs=2))
            t1pool = ctx.enter_context(tc.tile_pool(name="t1p", bufs=2))
            trpool = ctx.enter_context(tc.tile_pool(name="trp", bufs=6))
            outpool = ctx.enter_context(tc.tile_pool(name="outp", bufs=2))
            psum_pool = ctx.enter_context(tc.tile_pool(name="ps", bufs=4, space="PSUM"))

            ident = const_pool.tile([P, P], f32)
            nc.sync.dma_start(out=ident, in_=ident_d[:, :])
            # Warm-up matmul: its only dependency is the ident DMA, so the
            # (single) sync wait fp32 Matmult supports is that DMA. PE is
            # FIFO, so every later matmul sees loaded weights.
            warm = psum_pool.tile([P, 1], f32, tag="ps")
            nc.tensor.matmul(warm, ident, ident[:, 0:1], start=True, stop=True)
            # Prime the PSUM has_written bits over every address the ps-tag
            # slots cover: a start=False matmul only ACCUMULATES where the PE
            # has written since power-on (bit=1); elsewhere it overwrites,
            # which would drop the DVE-written half on first use of a bank.
            identb = ident[:, :]
            for _slot in range(4):
                pt = psum_pool.tile([P, g * piece_w], f32, tag="ps")
                ptb = pt[:, :]
                for half in range(0, g * piece_w, 512):
                    pw = min(512, g * piece_w - half)
                    rhs = _ap(identb, 0, [[0, pw]], bass)
                    pso = _ap(ptb, half, [[1, pw]], bass)
                    nc.tensor.matmul(pso, ident, rhs, start=True, stop=True,
                                     skip_group_check=True)

            mult = mybir.AluOpType.mult
            add = mybir.AluOpType.add
            sub = mybir.AluOpType.subtract

            for mt in range(n_mt):
                r0 = mt * P * g
                # ---- load z: z2 [P, 2, g, N] (c outer) ----
                z2 = zpool.tile([P, 2, g, N], f32)
                src_re = z_re_d[r0:r0 + P * g, :].rearrange("(p g) f -> p g f", g=g)
                src_im = z_im_d[r0:r0 + P * g, :].rearrange("(p g) f -> p g f", g=g)
                nc.sync.dma_start(out=z2[:, 0, :, :], in_=src_re)
                nc.sync.dma_start(out=z2[:, 1, :, :], in_=src_im)
                zb = z2[:, :, :, :]  # base AP; free dims [2*g*N] strides: c=g*N, g=N, f=1
                cZ, gZ = g * N, N

                # ---- w stage: w2 [P, 2, g, 15] ----
                w2 = wpool.tile([P, 2, g, len(WPAIRS)], f32)
                wb = w2[:, :, :, :]
                cW, gW = g * len(WPAIRS), len(WPAIRS)
                for i in range(N):
                    ti_ = N - i
                    off = WOFF[(i, i)]
                    # m1 = (zre_i, zim_i) bcast * (zre[i:], zim[i:]) -> [P, g, 2, ti]
                    t1 = t1pool.tile([P, g, 2, N], f32)
                    t1b = t1[:, :, :, :]
                    in0 = _ap(zb, i, [[gZ, g], [cZ, 2], [0, ti_]], bass)
                    in1 = _ap(zb, i, [[gZ, g], [cZ, 2], [1, ti_]], bass)
                    o1 = _ap(t1b, 0, [[2 * N, g], [N, 2], [1, ti_]], bass)
                    pick(2 * g * ti_).tensor_tensor(out=o1, in0=in0, in1=in1, op=mult)
                    # w_re[i block] = m1[c0] - m1[c1]
                    a0 = _ap(t1b, 0, [[2 * N, g], [1, ti_]], bass)
                    a1 = _ap(t1b, N, [[2 * N, g], [1, ti_]], bass)
                    ow = _ap(wb, off, [[gW, g], [1, ti_]], bass)
                    pick(g * ti_).tensor_tensor(out=ow, in0=a0, in1=a1, op=sub)
                    # m2 = (zim_i, zre_i) bcast * (zre[i:], zim[i:])
                    t2 = t1pool.tile([P, g, 2, N], f32, tag="t2")
                    t2b = t2[:, :, :, :]
                    in0r = _ap(zb, cZ + i, [[gZ, g], [-cZ, 2], [0, ti_]], bass)
                    o2 = _ap(t2b, 0, [[2 * N, g], [N, 2], [1, ti_]], bass)
                    pick(2 * g * ti_).tensor_tensor(out=o2, in0=in0r, in1=in1, op=mult)
                    a0 = _ap(t2b, 0, [[2 * N, g], [1, ti_]], bass)
                    a1 = _ap(t2b, N, [[2 * N, g], [1, ti_]], bass)
                    ow = _ap(wb, cW + off, [[gW, g], [1, ti_]], bass)
                    pick(g * ti_).tensor_tensor(out=ow, in0=a0, in1=a1, op=add)

                # ---- zz stage: zz3 [P, 3, g, 35] (re, im, negim) ----
                zz3 = zzpool.tile([P, 3, g, M], f32)
                zzb = zz3[:, :, :, :]
                cA, gA = g * M, M
                for (i, j) in WPAIRS:
                    tk = N - j
                    pr = WOFF[(i, j)]
                    zo = ZOFF[(i, j)]
                    # m3 = (wre, wim) bcast * (zre[j:], zim[j:])
                    t3 = t1pool.tile([P, g, 2, N], f32, tag="t3")
                    t3b = t3[:, :, :, :]
                    in0 = _ap(wb, pr, [[gW, g], [cW, 2], [0, tk]], bass)
                    in1 = _ap(zb, j, [[gZ, g], [cZ, 2], [1, tk]], bass)
                    o3 = _ap(t3b, 0, [[2 * N, g], [N, 2], [1, tk]], bass)
                    pick(2 * g * tk).tensor_tensor(out=o3, in0=in0, in1=in1, op=mult)
                    a0 = _ap(t3b, 0, [[2 * N, g], [1, tk]], bass)
                    a1 = _ap(t3b, N, [[2 * N, g], [1, tk]], bass)
                    oz = _ap(zzb, zo, [[gA, g], [1, tk]], bass)
                    pick(g * tk).tensor_tensor(out=oz, in0=a0, in1=a1, op=sub)
                    # m4 = (wim, wre) bcast * (zre[j:], zim[j:])
                    t4 = t1pool.tile([P, g, 2, N], f32, tag="t4")
                    t4b = t4[:, :, :, :]
                    in0r = _ap(wb, cW + pr, [[gW, g], [-cW, 2], [0, tk]], bass)
                    o4 = _ap(t4b, 0, [[2 * N, g], [N, 2], [1, tk]], bass)
                    pick(2 * g * tk).tensor_tensor(out=o4, in0=in0r, in1=in1, op=mult)
                    a0 = _ap(t4b, 0, [[2 * N, g], [1, tk]], bass)
                    a1 = _ap(t4b, N, [[2 * N, g], [1, tk]], bass)
                    oz = _ap(zzb, cA + zo, [[gA, g], [1, tk]], bass)
                    pick(g * tk).tensor_tensor(out=oz, in0=a0, in1=a1, op=add)
                # negim slot: zz3[:,2] = -zz3[:,1]
                src = _ap(zzb, cA, [[gA, g], [1, M]], bass)
                dst = _ap(zzb, 2 * cA, [[gA, g], [1, M]], bass)
                eng_load["g"] += 95 + 1.1 * g * M
                nc.gpsimd.tensor_scalar_mul(out=dst, in0=src, scalar1=-1.0)

                # ---- products: DVE half -> PSUM, PE accumulates SBUF half,
                # ---- ACT drains PSUM -> out chunk, chunk DMA'd out
                for (colbase, cols, blist) in chunks:
                    outc = outpool.tile([P, g, chunk_cols], f32)
                    ocb = outc[:, :, :]
                    gO = chunk_cols

                    def prod_aps(kind, p, t0, w):
                        """(inA0, inA1, inB0, inB1) for a block sub-range
                        [t0, t0+w) of block p's products (A half -> PSUM via
                        DVE; B half -> SBUF for the PE accumulate)."""
                        if kind == "re":
                            # A: Rp * R[p+t0:] ; B: Ip * I[p+t0:]
                            return (_ap(zzb, p, [[gA, g], [0, w]], bass),
                                    _ap(zzb, p + t0, [[gA, g], [1, w]], bass),
                                    _ap(zzb, cA + p, [[gA, g], [0, w]], bass),
                                    _ap(zzb, cA + p + t0, [[gA, g], [1, w]], bass))
                        # A: Ip * R[p+1+t0:] ; B: Rp * (-I[p+1+t0:])
                        return (_ap(zzb, cA + p, [[gA, g], [0, w]], bass),
                                _ap(zzb, p + 1 + t0, [[gA, g], [1, w]], bass),
                                _ap(zzb, p, [[gA, g], [0, w]], bass),
                                _ap(zzb, 2 * cA + p + 1 + t0, [[gA, g], [1, w]], bass))

                    for (kind, p, cb, w) in blist:
                        if w <= 8:
                            # small block: pure DVE/GPS, no PSUM round-trip.
                            # one c-packed mult for both halves, then add.
                            tsm = t1pool.tile([P, g, 2, 8], f32, tag="tsm")
                            tsb = tsm[:, :, :, :]
                            iA0, iA1, iB0, iB1 = prod_aps(kind, p, 0, w)
                            in0 = bass.AP(tensor=iA0.tensor, offset=iA0.offset,
                                          ap=[iA0.ap[0], iA0.ap[1],
                                              [iB0.offset - iA0.offset, 2], iA0.ap[2]])
                            in1 = bass.AP(tensor=iA1.tensor, offset=iA1.offset,
                                          ap=[iA1.ap[0], iA1.ap[1],
                                              [iB1.offset - iA1.offset, 2], iA1.ap[2]])
                            om = _ap(tsb, 0, [[16, g], [8, 2], [1, w]], bass)
                            pick(2 * g * w).tensor_tensor(out=om, in0=in0, in1=in1, op=mult)
                            oc = _ap(ocb, cb, [[gO, g], [1, w]], bass)
                            s0 = _ap(tsb, 0, [[16, g], [1, w]], bass)
                            s1 = _ap(tsb, 8, [[16, g], [1, w]], bass)
                            pick(g * w).tensor_tensor(out=oc, in0=s0, in1=s1, op=add)
                            continue
                        # big block: split into <=32-col pieces (2 PSUM banks)
                        t0 = 0
                        while t0 < w:
                            ww = min(piece_w, w - t0)
                            pst = psum_pool.tile([P, g, ww], f32, tag="ps")
                            psb = pst[:, :, :]
                            tr = trpool.tile([P, g, piece_w], f32, tag="tr")
                            trb = tr[:, :, :]
                            inA0, inA1, inB0, inB1 = prod_aps(kind, p, t0, ww)
                            outA = _ap(psb, 0, [[ww, g], [1, ww]], bass)
                            eng_load["v"] += 250 + 1.042 * g * ww
                            nc.vector.tensor_tensor(out=outA, in0=inA0, in1=inA1, op=mult)
                            outB = _ap(trb, 0, [[ww, g], [1, ww]], bass)  # packed flat
                            pick(g * ww).tensor_tensor(out=outB, in0=inB0, in1=inB1, op=mult)
                            # PE: accumulate sbuf half onto psum, 512-elem flat chunks
                            flat = g * ww
                            f0 = 0
                            while f0 < flat:
                                tcw = min(512, flat - f0)
                                rhs = _ap(trb, f0, [[1, tcw]], bass)
                                pso = _ap(psb, f0, [[1, tcw]], bass)
                                nc.tensor.matmul(pso, ident, rhs, start=False, stop=True,
                                                 skip_group_check=True)
                                f0 += tcw
                            # ACT: drain psum piece -> out chunk columns
                            oc = _ap(ocb, cb + t0, [[gO, g], [1, ww]], bass)
                            nc.scalar.copy(out=oc, in_=_ap(psb, 0, [[ww, g], [1, ww]], bass))
                            t0 += ww
                    # DMA chunk out
                    dst = out_d[r0:r0 + P * g, colbase:colbase + cols].rearrange(
                        "(p g) f -> p g f", g=g)
                    nc.sync.dma_start(out=dst, in_=_ap(ocb, 0, [[gO, g], [1, cols]], bass))

    nc.finalize()
    return nc


_CACHED = {}


def _get_nc():
    if "nc" not in _CACHED:
        _CACHED["nc"] = build_bass()
    return _CACHED["nc"]


def kernel(z_re, z_im):
    from concourse.bass_utils import run_bass_kernel_spmd

    z_re = np.ascontiguousarray(np.asarray(z_re, dtype=np.float32))
    z_im = np.ascontiguousarray(np.asarray(z_im, dtype=np.float32))
    assert z_re.shape == (B_FULL, N), z_re.shape

    nc = _get_nc()
    ident = np.eye(P, dtype=np.float32)
    in_maps = []
    for c in range(NC):
        sl = slice(c * B_LOCAL, (c + 1) * B_LOCAL)
        in_maps.append({
            "z_re": np.ascontiguousarray(z_re[sl]),
            "z_im": np.ascontiguousarray(z_im[sl]),
            "ident": ident,
        })
    res = run_bass_kernel_spmd(nc, in_maps, core_ids=list(range(NC)))
    return np.concatenate([res.results[c]["out"] for c in range(NC)], axis=0)



# revision 2
# speedup vs baseline: 1.0043x; 1.0043x over previous
"""Trainium2 Bass kernel v2 for nn_Bihomogeneous_k3 (fp16 feature-major).

Math per batch row (complex z dim 5 given as z_re/z_im):
  zz[m]  = z_i z_j z_k for 35 triples i<=j<=k (lex order)
  prod   = zz[p] * conj(zz[q]) for 630 pairs p<=q
  out    = [Re(prod) (630) | Im(prod) strict p<q (595)] -> [B, 1225]

Distribution: pure data parallel over 8 cores (batch sharded).

Per-core design (B_local=16384, P=128 partitions, G=64 rows/partition/mt,
2 megatiles). All pair-product compute in fp16, FEATURE-MAJOR layout
[P, c, feat, g]: every DVE op's innermost dim is the packed g-run, which
qualifies fp16 TensorTensor for the 2x_1p DVE perf mode (0.52 ns/elem).

Gauss 3-mult complex products per p-block (q in [p,34]):
  slabs: S = R+I, Sn = -S, D = I-R   (R/I = re/im of zz)
  k1 = S[p]*R[q]; k2 = Rp*Sn[q]; k3n = Ip*D[q]
  re(p,q) = k1 + k3n  (q>=p; q=p gives the diagonal Rp^2+Ip^2 for free)
  im(p,q) = k1 + k2   (q>p)
Adds are routed either:
  - PSUM: two fp16 identity-weight matmuls (start=True then accumulate)
    into 2048-elem PSUM slots; ACT drains (and transposes to batch-major,
    converting f32->fp16) into the output chunk, or
  - direct: small tail blocks (w<=WC) as one transposing TT add on GPS/DVE.
Output chunks [P, g, ~306 cols] fp16 are DMA'd out (>=512B contiguous runs
-> full 360B/ns cost-model DMA rate); host concatenates + casts to f32.
"""
import sys

sys.path.insert(0, "/opt/trn_rl_repo")

import numpy as np

N = 5
NC = 8
B_FULL = 131072
B_LOCAL = B_FULL // NC
P = 128
G = 64
N_MT = B_LOCAL // (P * G)  # 2

# ---- index tables ----
WPAIRS = [(i, j) for i in range(N) for j in range(i, N)]  # 15 lex
WOFF = {}
_o = 0
for (i, j) in WPAIRS:
    WOFF[(i, j)] = _o
    _o += 1
ZOFF = {}
_o = 0
for (i, j) in WPAIRS:
    ZOFF[(i, j)] = _o
    _o += N - j
M = _o
assert M == 35

REOFF = np.concatenate([[0], np.cumsum([M - p for p in range(M)])]).astype(int)
IMOFF = np.concatenate([[0], np.cumsum([M - 1 - p for p in range(M - 1)])]).astype(int)
N_RE = int(REOFF[M])      # 630
N_IM = int(IMOFF[M - 1])  # 595
N_OUT = N_RE + N_IM       # 1225

WC = 14          # blocks with add-width <= WC go the direct (no-PSUM) route
SPLIT_P = 11     # stream chunk boundary: blocks p < SPLIT_P | p >= SPLIT_P
QCAP = 20        # max q-segment width (caps k-tile SBUF footprint)
DBG = set()      # timing knockouts: no_dma_out / no_drain / single_mm / no_prod
KT_BUFS = 3      # k-tile ring depth
ZZ_BUFS = 1      # zz/gs double buffering (2 = overlap next mt's zz stage)
SLOT = 2048      # psum slot size (f32 elems); SLOT * PS_BUFS <= 4096
PS_BUFS = 2
INTERLEAVE = False


def _make_chunks(kind):
    """Two column chunks for one output stream ('re' or 'im'), split at
    p == SPLIT_P. All chunk widths land in [256, 330] so fp16 DMA runs are
    >= 512B. Returns [(colbase, cols, [(p, cb_in_chunk, w), ...]), ...] with
    colbase in FULL output coordinates (im stream offset by N_RE)."""
    if kind == "re":
        blocks = [(p, int(REOFF[p]), M - p) for p in range(M)]
    else:
        blocks = [(p, N_RE + int(IMOFF[p]), M - 1 - p) for p in range(M - 1)]
    chunks = []
    cur, base, cols = [], blocks[0][1], 0
    for p, cb, w in blocks:
        if p == SPLIT_P and cur:
            chunks.append((base, cols, cur))
            base, cols, cur = cb, 0, []
        cur.append((p, cb - base, w))
        cols += w
    if cur:
        chunks.append((base, cols, cur))
    assert all(256 <= c[1] <= 330 for c in chunks), [c[1] for c in chunks]
    return chunks


def _ap(base_ap, offset_elems, dims, bassmod):
    return bassmod.AP(tensor=base_ap.tensor, offset=base_ap.offset + offset_elems,
                      ap=[list(base_ap.ap[0])] + [list(d) for d in dims])


def build_bass(g=G, n_mt=N_MT):
    import concourse.bacc as bacc
    import concourse.bass as bass
    import concourse.tile as tile
    from concourse import mybir
    from contextlib import ExitStack

    f32 = mybir.dt.float32
    f16 = mybir.dt.float16
    b_local = P * g * n_mt

    nc = bacc.Bacc(None)
    z_re_d = nc.dram_tensor("z_re", [b_local, N], f32, kind="ExternalInput")
    z_im_d = nc.dram_tensor("z_im", [b_local, N], f32, kind="ExternalInput")
    ident_d = nc.dram_tensor("ident", [P, P], f16, kind="ExternalInput")
    out_d = nc.dram_tensor("out", [b_local, N_OUT], f16, kind="ExternalOutput")

    mult = mybir.AluOpType.mult
    add = mybir.AluOpType.add
    sub = mybir.AluOpType.subtract

    # greedy DVE/GPS balance (ns; fd = free elems). DVE fp16 2x_1p; GPS eff.
    eng = {"v": 0.0, "g": 0.0}

    def pick(fd, dve_rate):
        cv = eng["v"] + 100 + dve_rate * fd
        cg = eng["g"] + 131 + 1.99 * fd
        if cv <= cg:
            eng["v"] = cv
            return nc.vector
        eng["g"] = cg
        return nc.gpsimd

    with tile.TileContext(nc) as tc:
        with ExitStack() as ctx:
            const_pool = ctx.enter_context(tc.tile_pool(name="const", bufs=1))
            z32p = ctx.enter_context(tc.tile_pool(name="z32p", bufs=1))
            z16p = ctx.enter_context(tc.tile_pool(name="z16p", bufs=1))
            zzp = ctx.enter_context(tc.tile_pool(name="zzp", bufs=ZZ_BUFS))
            wpool = ctx.enter_context(tc.tile_pool(name="wp", bufs=1))
            gsp = ctx.enter_context(tc.tile_pool(name="gsp", bufs=ZZ_BUFS))
            tpool = ctx.enter_context(tc.tile_pool(name="tp", bufs=1))
            ktp = ctx.enter_context(tc.tile_pool(name="ktp", bufs=KT_BUFS))
            outp = ctx.enter_context(tc.tile_pool(name="outp", bufs=1))
            psp = ctx.enter_context(tc.tile_pool(name="psp", bufs=PS_BUFS,
                                                 space="PSUM"))

            ident = const_pool.tile([P, P], f16)
            nc.sync.dma_start(out=ident, in_=ident_d[:, :])
            # Warm-up matmul (weight load); uses the regular psum slot ring so
            # PSUM stays at exactly 2 x 2048 f32 = 16KB/partition.
            warm = psp.tile([P, SLOT], f32, tag="ps")
            nc.tensor.matmul(warm[:, 0:1], ident, ident[:, 0:1], start=True,
                             stop=True)

            for mt in range(n_mt):
                r0 = mt * P * g

                # ---- load z batch-major, convert+transpose to feature-major
                z32 = z32p.tile([P, 2, g, N], f32)
                nc.sync.dma_start(
                    out=z32[:, 0, :, :],
                    in_=z_re_d[r0:r0 + P * g, :].rearrange("(p g) f -> p g f", g=g))
                nc.sync.dma_start(
                    out=z32[:, 1, :, :],
                    in_=z_im_d[r0:r0 + P * g, :].rearrange("(p g) f -> p g f", g=g))
                z16 = z16p.tile([P, 2, N, g], f16)
                z32b = z32[:, :, :, :]
                z16b = z16[:, :, :, :]
                # iterate (c, i, gi): in (c, gi, i) strides, out packed
                nc.vector.tensor_scalar_mul(
                    out=_ap(z16b, 0, [[N * g, 2], [g, N], [1, g]], bass),
                    in0=_ap(z32b, 0, [[N * g, 2], [1, N], [N, g]], bass),
                    scalar1=1.0)
                eng["v"] += 100 + 1.042 * 2 * N * g
                cZ = N * g

                # ---- w stage: w16 [P, 2, 15, g] ----
                w16 = wpool.tile([P, 2, len(WPAIRS), g], f16)
                w16b = w16[:, :, :, :]
                wbase = 0
                cA = M * g
                cW = len(WPAIRS) * g
                for i in range(N):
                    ti = N - i
                    off = WOFF[(i, i)]
                    t1 = tpool.tile([P, 2, N, g], f16, tag="t1")
                    t1b = t1[:, :, :, :]
                    # m1 = (zre_i, zim_i) bcast * (zre[i:], zim[i:])
                    pick(2 * ti * g, 0.53).tensor_tensor(
                        out=_ap(t1b, 0, [[cZ, 2], [g, ti], [1, g]], bass),
                        in0=_ap(z16b, i * g, [[cZ, 2], [0, ti], [1, g]], bass),
                        in1=_ap(z16b, i * g, [[cZ, 2], [g, ti], [1, g]], bass),
                        op=mult)
                    pick(ti * g, 0.53).tensor_tensor(
                        out=_ap(w16b, wbase + off * g, [[g, ti], [1, g]], bass),
                        in0=_ap(t1b, 0, [[g, ti], [1, g]], bass),
                        in1=_ap(t1b, cZ, [[g, ti], [1, g]], bass),
                        op=sub)
                    # m2 = (zim_i, zre_i) bcast * (zre[i:], zim[i:])
                    t2 = tpool.tile([P, 2, N, g], f16, tag="t2")
                    t2b = t2[:, :, :, :]
                    pick(2 * ti * g, 0.53).tensor_tensor(
                        out=_ap(t2b, 0, [[cZ, 2], [g, ti], [1, g]], bass),
                        in0=_ap(z16b, cZ + i * g, [[-cZ, 2], [0, ti], [1, g]], bass),
                        in1=_ap(z16b, i * g, [[cZ, 2], [g, ti], [1, g]], bass),
                        op=mult)
                    pick(ti * g, 0.53).tensor_tensor(
                        out=_ap(w16b, cW + off * g, [[g, ti], [1, g]], bass),
                        in0=_ap(t2b, 0, [[g, ti], [1, g]], bass),
                        in1=_ap(t2b, cZ, [[g, ti], [1, g]], bass),
                        op=add)

                # ---- zz stage: zz16 [P, 2, 35, g] (R, I) ----
                zz16 = zzp.tile([P, 2, M, g], f16)
                zzb = zz16[:, :, :, :]
                zzR = 0
                zzI = cA
                for (i, j) in WPAIRS:
                    tk = N - j
                    pr = WOFF[(i, j)]
                    zo = ZOFF[(i, j)]
                    t3 = tpool.tile([P, 2, N, g], f16, tag="t1")
                    t3b = t3[:, :, :, :]
                    pick(2 * tk * g, 0.53).tensor_tensor(
                        out=_ap(t3b, 0, [[cZ, 2], [g, tk], [1, g]], bass),
                        in0=_ap(w16b, pr * g, [[cW, 2], [0, tk], [1, g]],
                                bass),
                        in1=_ap(z16b, j * g, [[cZ, 2], [g, tk], [1, g]], bass),
                        op=mult)
                    pick(tk * g, 0.53).tensor_tensor(
                        out=_ap(zzb, zzR + zo * g, [[g, tk], [1, g]], bass),
                        in0=_ap(t3b, 0, [[g, tk], [1, g]], bass),
                        in1=_ap(t3b, cZ, [[g, tk], [1, g]], bass),
                        op=sub)
                    t4 = tpool.tile([P, 2, N, g], f16, tag="t2")
                    t4b = t4[:, :, :, :]
                    pick(2 * tk * g, 0.53).tensor_tensor(
                        out=_ap(t4b, 0, [[cZ, 2], [g, tk], [1, g]], bass),
                        in0=_ap(w16b, cW + pr * g,
                                [[-cW, 2], [0, tk], [1, g]], bass),
                        in1=_ap(z16b, j * g, [[cZ, 2], [g, tk], [1, g]], bass),
                        op=mult)
                    pick(tk * g, 0.53).tensor_tensor(
                        out=_ap(zzb, zzI + zo * g, [[g, tk], [1, g]], bass),
                        in0=_ap(t4b, 0, [[g, tk], [1, g]], bass),
                        in1=_ap(t4b, cZ, [[g, tk], [1, g]], bass),
                        op=add)

                # ---- Gauss slabs: gs [P, 3, 35, g] = S, Sn, D ----
                gs = gsp.tile([P, 3, M, g], f16)
                gsb = gs[:, :, :, :]
                pick(cA, 0.53).tensor_tensor(
                    out=_ap(gsb, 0, [[1, cA]], bass),
                    in0=_ap(zzb, 0, [[1, cA]], bass),
                    in1=_ap(zzb, cA, [[1, cA]], bass), op=add)        # S = R+I
                nc.vector.tensor_scalar_mul(
                    out=_ap(gsb, cA, [[1, cA]], bass),
                    in0=_ap(gsb, 0, [[1, cA]], bass), scalar1=-1.0)   # Sn = -S
                eng["v"] += 100 + 0.27 * cA
                pick(cA, 0.53).tensor_tensor(
                    out=_ap(gsb, 2 * cA, [[1, cA]], bass),
                    in0=_ap(zzb, cA, [[1, cA]], bass),
                    in1=_ap(zzb, 0, [[1, cA]], bass), op=sub)         # D = I-R

                # ---- products: iterate p once; k-tiles feed BOTH streams ----
                # Interleaved p order (psum-heavy low-p alternating with
                # direct-route high-p) keeps PE/ACT and GPS busy concurrently
                # instead of in two serialized phases.
                # Out tile per (kind, chunk-index) tag, opened lazily, closed
                # when every block column has been written (static fill map).
                st = {}
                for kind in ("re", "im"):
                    chunks = _make_chunks(kind)
                    blkmap = {}  # p -> (ci, cb_in_chunk)
                    for ci, (_cb, _cols, bl) in enumerate(chunks):
                        for (p, cb, w) in bl:
                            blkmap[p] = (ci, cb)
                    st[kind] = {"chunks": chunks, "blk": blkmap,
                                "tile": {}, "pfx": {},
                                "left": {ci: c[1] for ci, c in
                                         enumerate(chunks)}}

                slot = None  # [ps_ap, off, segs]; seg=[ocb, cw, oc0, po, nc]

                def drain():
                    nonlocal slot
                    if slot is None:
                        return
                    pb, _soff, segs = slot
                    if "no_drain" not in DBG:
                        for (socb, cw, oc0, po, ncols) in segs:
                            nc.scalar.copy(
                                out=_ap(socb, oc0, [[cw, g], [1, ncols]], bass),
                                in_=_ap(pb, po, [[1, g], [g, ncols]], bass))
                    slot = None

                def open_chunk(kind, ci):
                    s = st[kind]
                    cols = s["chunks"][ci][1]
                    t = outp.tile([P, g, cols], f16, tag=f"oc_{kind}{ci}")
                    s["tile"][ci] = t[:, :, :]

                def dma_cols(kind, ci, c0, c1):
                    if "no_dma_out" in DBG or c1 <= c0:
                        return
                    s = st[kind]
                    colbase, cols, _bl = s["chunks"][ci]
                    dst = out_d[r0:r0 + P * g,
                                colbase + c0:colbase + c1].rearrange(
                        "(p g) f -> p g f", g=g)
                    nc.sync.dma_start(
                        out=dst,
                        in_=_ap(s["tile"][ci], c0, [[cols, g], [1, c1 - c0]],
                                bass))

                def close_chunk(kind, ci):
                    s = st[kind]
                    drain()  # open slot may reference this chunk's tile
                    cols = s["chunks"][ci][1]
                    dma_cols(kind, ci, s["pfx"].get(ci, 0), cols)
                    del s["tile"][ci]

                def emit(kind, p, seg_off, kAb, offA, kBb, offB, w):
                    nonlocal slot
                    s = st[kind]
                    ci, cb0 = s["blk"][p]
                    if ci not in s["tile"]:
                        open_chunk(kind, ci)
                    cb = cb0 + seg_off
                    ocb = s["tile"][ci]
                    cw = s["chunks"][ci][1]
                    if w <= WC:
                        pick(w * g, 1.062).tensor_tensor(
                            out=_ap(ocb, cb, [[cw, g], [1, w]], bass),
                            in0=_ap(kAb, offA, [[1, g], [g, w]], bass),
                            in1=_ap(kBb, offB, [[1, g], [g, w]], bass),
                            op=add)
                    else:
                        # block-atomic slot: no mid-block slot boundary, so
                        # every block is one ACT drain segment
                        if slot is not None and slot[1] + w * g > SLOT:
                            drain()
                        if slot is None:
                            pst = psp.tile([P, SLOT], f32, tag="ps")
                            slot = [pst[:, :], 0, []]
                        pb, soff, segs = slot
                        segs.append([ocb, cw, cb, soff, w])
                        rem = w * g
                        fA, fB = offA, offB
                        while rem > 0:
                            n = min(rem, 512 - soff % 512)
                            pso = _ap(pb, soff, [[1, n]], bass)
                            nc.tensor.matmul(
                                pso, ident, _ap(kAb, fA, [[1, n]], bass),
                                start=True, stop=False, skip_group_check=True)
                            if "single_mm" not in DBG:
                                nc.tensor.matmul(
                                    pso, ident, _ap(kBb, fB, [[1, n]], bass),
                                    start=False, stop=True,
                                    skip_group_check=True)
                            soff += n
                            fA += n
                            fB += n
                            rem -= n
                        slot[1] = soff
                        if slot[1] == SLOT:
                            drain()
                    s["left"][ci] -= w
                    left = s["left"][ci]
                    cols = s["chunks"][ci][1]
                    if left == 0:
                        close_chunk(kind, ci)
                    elif ci not in s["pfx"] and left <= TAILC and \
                            cols - left >= 256:
                        # fire the bulk of the chunk early; only a small
                        # suffix DMA remains at close (shrinks the end-of-
                        # kernel DMA tail). Drain first so psum-routed cols
                        # are in SBUF before the DMA reads them.
                        drain()
                        dma_cols(kind, ci, 0, cols - left)
                        s["pfx"][ci] = cols - left

                if INTERLEAVE:
                    lowp = [p for p in range(M) if M - p > WC]
                    highp = [p for p in range(M - 1, -1, -1) if M - p <= WC]
                    p_order = []
                    for i in range(max(len(lowp), len(highp))):
                        if i < len(lowp):
                            p_order.append(lowp[i])
                        if i < len(highp):
                            p_order.append(highp[i])
                else:
                    p_order = list(range(M))

                cK = QCAP * g
                for p in p_order:
                    q0 = p
                    while q0 < M:
                        ws = min(QCAP, M - q0)  # segment [q0, q0+ws)
                        kt = ktp.tile([P, 3, QCAP, g], f16, tag="kt")
                        ktb = kt[:, :, :, :]
                        # fused: (k1, k2, k3n) = (S[p], Rp, Ip)*(R, Sn, D)[q0:]
                        pick(3 * ws * g, 0.53).tensor_tensor(
                            out=_ap(ktb, 0, [[cK, 3], [g, ws], [1, g]], bass),
                            in0=_ap(zzgsb, SL_S * cA + p * g,
                                    [[cA, 3], [0, ws], [1, g]], bass),
                            in1=_ap(zzgsb, SL_R * cA + q0 * g,
                                    [[2 * cA, 3], [g, ws], [1, g]], bass),
                            op=mult)
                        # re = k1 + k3n over q in [q0, q0+ws)
                        emit("re", p, q0 - p, ktb, 0, ktb, 2 * cK, ws)
                        # im = k1 + k2 over q > p
                        sk = g if q0 == p else 0  # skip diagonal col
                        if ws * g - sk > 0:
                            emit("im", p, q0 + sk // g - (p + 1), ktb, sk,
                                 ktb, cK + sk, ws - sk // g)
                        q0 += ws
                for kind in ("re", "im"):
                    assert not st[kind]["tile"], (kind, st[kind]["left"])

    nc.finalize()
    return nc


_CACHED = {}


def _get_nc():
    if "nc" not in _CACHED:
        _CACHED["nc"] = build_bass()
    return _CACHED["nc"]


def kernel(z_re, z_im):
    from concourse.bass_utils import run_bass_kernel_spmd

    z_re = np.ascontiguousarray(np.asarray(z_re, dtype=np.float32))
    z_im = np.ascontiguousarray(np.asarray(z_im, dtype=np.float32))
    assert z_re.shape == (B_FULL, N), z_re.shape

    nc = _get_nc()
    ident = np.eye(P, dtype=np.float16)
    in_maps = []
    for c in range(NC):
        sl = slice(c * B_LOCAL, (c + 1) * B_LOCAL)
        in_maps.append({
            "z_re": np.ascontiguousarray(z_re[sl]),
            "z_im": np.ascontiguousarray(z_im[sl]),
            "ident": ident,
        })
    res = run_bass_kernel_spmd(nc, in_maps, core_ids=list(range(NC)))
    out = np.concatenate([res.results[c]["out"] for c in range(NC)], axis=0)
    return out.astype(np.float32)


# revision 3
# speedup vs baseline: 1.0736x; 1.0691x over previous
"""Trainium2 Bass kernel v2 for nn_Bihomogeneous_k3 (fp16 feature-major).

Math per batch row (complex z dim 5 given as z_re/z_im):
  zz[m]  = z_i z_j z_k for 35 triples i<=j<=k (lex order)
  prod   = zz[p] * conj(zz[q]) for 630 pairs p<=q
  out    = [Re(prod) (630) | Im(prod) strict p<q (595)] -> [B, 1225]

Distribution: pure data parallel over 8 cores (batch sharded).

Per-core design (B_local=16384, P=128 partitions, G=64 rows/partition/mt,
2 megatiles). All pair-product compute in fp16, FEATURE-MAJOR layout
[P, c, feat, g]: every DVE op's innermost dim is the packed g-run, which
qualifies fp16 TensorTensor for the 2x_1p DVE perf mode (0.52 ns/elem).

Gauss 3-mult complex products per p-block (q in [p,34]):
  slabs: S = R+I, Sn = -S, D = I-R   (R/I = re/im of zz)
  k1 = S[p]*R[q]; k2 = Rp*Sn[q]; k3n = Ip*D[q]
  re(p,q) = k1 + k3n  (q>=p; q=p gives the diagonal Rp^2+Ip^2 for free)
  im(p,q) = k1 + k2   (q>p)
Adds are routed either:
  - PSUM: two fp16 identity-weight matmuls (start=True then accumulate)
    into 2048-elem PSUM slots; ACT drains (and transposes to batch-major,
    converting f32->fp16) into the output chunk, or
  - direct: small tail blocks (w<=WC) as one transposing TT add on GPS/DVE.
Output chunks [P, g, ~306 cols] fp16 are DMA'd out (>=512B contiguous runs
-> full 360B/ns cost-model DMA rate); host concatenates + casts to f32.
"""
import sys

sys.path.insert(0, "/opt/trn_rl_repo")

import numpy as np

N = 5
NC = 8
B_FULL = 131072
B_LOCAL = B_FULL // NC
P = 128
G = 64
N_MT = B_LOCAL // (P * G)  # 2

# ---- index tables ----
WPAIRS = [(i, j) for i in range(N) for j in range(i, N)]  # 15 lex
WOFF = {}
_o = 0
for (i, j) in WPAIRS:
    WOFF[(i, j)] = _o
    _o += 1
ZOFF = {}
_o = 0
for (i, j) in WPAIRS:
    ZOFF[(i, j)] = _o
    _o += N - j
M = _o
assert M == 35

REOFF = np.concatenate([[0], np.cumsum([M - p for p in range(M)])]).astype(int)
IMOFF = np.concatenate([[0], np.cumsum([M - 1 - p for p in range(M - 1)])]).astype(int)
N_RE = int(REOFF[M])      # 630
N_IM = int(IMOFF[M - 1])  # 595
N_OUT = N_RE + N_IM       # 1225

WC = 14          # blocks with add-width <= WC go the direct (no-PSUM) route
SPLIT_P = 10     # stream chunk boundary: blocks p < SPLIT_P | p >= SPLIT_P
QCAP = 20        # max q-segment width (caps k-tile SBUF footprint)
DBG = set()      # timing knockouts: no_dma_out / no_drain / single_mm / no_prod
KT_BUFS = 3      # k-tile ring depth
ZZ_BUFS = 1      # zz/gs double buffering (2 = overlap next mt's zz stage)
SLOT = 2048      # psum slot size (f32 elems); SLOT * PS_BUFS <= 4096
PS_BUFS = 2
INTERLEAVE = False


def _make_chunks(kind):
    """Two column chunks for one output stream ('re' or 'im'), split at
    p == SPLIT_P. All chunk widths land in [256, 330] so fp16 DMA runs are
    >= 512B. Returns [(colbase, cols, [(p, cb_in_chunk, w), ...]), ...] with
    colbase in FULL output coordinates (im stream offset by N_RE)."""
    if kind == "re":
        blocks = [(p, int(REOFF[p]), M - p) for p in range(M)]
    else:
        blocks = [(p, N_RE + int(IMOFF[p]), M - 1 - p) for p in range(M - 1)]
    chunks = []
    cur, base, cols = [], blocks[0][1], 0
    for p, cb, w in blocks:
        if p == SPLIT_P and cur:
            chunks.append((base, cols, cur))
            base, cols, cur = cb, 0, []
        cur.append((p, cb - base, w))
        cols += w
    if cur:
        chunks.append((base, cols, cur))
    assert all(256 <= c[1] <= 330 for c in chunks), [c[1] for c in chunks]
    return chunks


def _ap(base_ap, offset_elems, dims, bassmod):
    return bassmod.AP(tensor=base_ap.tensor, offset=base_ap.offset + offset_elems,
                      ap=[list(base_ap.ap[0])] + [list(d) for d in dims])


def build_bass(g=G, n_mt=N_MT):
    import concourse.bacc as bacc
    import concourse.bass as bass
    import concourse.tile as tile
    from concourse import mybir
    from contextlib import ExitStack

    f32 = mybir.dt.float32
    f16 = mybir.dt.float16
    b_local = P * g * n_mt

    nc = bacc.Bacc(None)
    z_re_d = nc.dram_tensor("z_re", [b_local, N], f32, kind="ExternalInput")
    z_im_d = nc.dram_tensor("z_im", [b_local, N], f32, kind="ExternalInput")
    ident_d = nc.dram_tensor("ident", [P, P], f16, kind="ExternalInput")
    out_d = nc.dram_tensor("out", [b_local, N_OUT], f16, kind="ExternalOutput")

    mult = mybir.AluOpType.mult
    add = mybir.AluOpType.add
    sub = mybir.AluOpType.subtract

    # greedy DVE/GPS balance (ns; fd = free elems). DVE fp16 2x_1p; GPS eff.
    eng = {"v": 0.0, "g": 0.0}

    def pick(fd, dve_rate):
        cv = eng["v"] + 100 + dve_rate * fd
        cg = eng["g"] + 131 + 2.3 * fd
        if cv <= cg:
            eng["v"] = cv
            return nc.vector
        eng["g"] = cg
        return nc.gpsimd

    with tile.TileContext(nc) as tc:
        with ExitStack() as ctx:
            const_pool = ctx.enter_context(tc.tile_pool(name="const", bufs=1))
            z32p = ctx.enter_context(tc.tile_pool(name="z32p", bufs=1))
            z16p = ctx.enter_context(tc.tile_pool(name="z16p", bufs=1))
            zzp = ctx.enter_context(tc.tile_pool(name="zzp", bufs=ZZ_BUFS))
            wpool = ctx.enter_context(tc.tile_pool(name="wp", bufs=1))
            gsp = ctx.enter_context(tc.tile_pool(name="gsp", bufs=ZZ_BUFS))
            tpool = ctx.enter_context(tc.tile_pool(name="tp", bufs=1))
            ktp = ctx.enter_context(tc.tile_pool(name="ktp", bufs=KT_BUFS))
            outp = ctx.enter_context(tc.tile_pool(name="outp", bufs=1))
            psp = ctx.enter_context(tc.tile_pool(name="psp", bufs=PS_BUFS,
                                                 space="PSUM"))

            ident = const_pool.tile([P, P], f16)
            nc.sync.dma_start(out=ident, in_=ident_d[:, :])
            # Warm-up matmul (weight load); uses the regular psum slot ring so
            # PSUM stays at exactly 2 x 2048 f32 = 16KB/partition.
            warm = psp.tile([P, SLOT], f32, tag="ps")
            nc.tensor.matmul(warm[:, 0:1], ident, ident[:, 0:1], start=True,
                             stop=True)

            for mt in range(n_mt):
                r0 = mt * P * g

                # ---- load z batch-major, convert+transpose to feature-major
                z32 = z32p.tile([P, 2, g, N], f32)
                nc.sync.dma_start(
                    out=z32[:, 0, :, :],
                    in_=z_re_d[r0:r0 + P * g, :].rearrange("(p g) f -> p g f", g=g))
                nc.sync.dma_start(
                    out=z32[:, 1, :, :],
                    in_=z_im_d[r0:r0 + P * g, :].rearrange("(p g) f -> p g f", g=g))
                z16 = z16p.tile([P, 2, N, g], f16)
                z32b = z32[:, :, :, :]
                z16b = z16[:, :, :, :]
                # iterate (c, i, gi): in (c, gi, i) strides, out packed
                nc.vector.tensor_scalar_mul(
                    out=_ap(z16b, 0, [[N * g, 2], [g, N], [1, g]], bass),
                    in0=_ap(z32b, 0, [[N * g, 2], [1, N], [N, g]], bass),
                    scalar1=1.0)
                eng["v"] += 100 + 1.042 * 2 * N * g
                cZ = N * g

                # ---- w stage: w16 [P, 2, 15, g] ----
                w16 = wpool.tile([P, 2, len(WPAIRS), g], f16)
                w16b = w16[:, :, :, :]
                wbase = 0
                cA = M * g
                cW = len(WPAIRS) * g
                for i in range(N):
                    ti = N - i
                    off = WOFF[(i, i)]
                    t1 = tpool.tile([P, 2, N, g], f16, tag="t1")
                    t1b = t1[:, :, :, :]
                    # m1 = (zre_i, zim_i) bcast * (zre[i:], zim[i:])
                    pick(2 * ti * g, 0.53).tensor_tensor(
                        out=_ap(t1b, 0, [[cZ, 2], [g, ti], [1, g]], bass),
                        in0=_ap(z16b, i * g, [[cZ, 2], [0, ti], [1, g]], bass),
                        in1=_ap(z16b, i * g, [[cZ, 2], [g, ti], [1, g]], bass),
                        op=mult)
                    pick(ti * g, 0.53).tensor_tensor(
                        out=_ap(w16b, wbase + off * g, [[g, ti], [1, g]], bass),
                        in0=_ap(t1b, 0, [[g, ti], [1, g]], bass),
                        in1=_ap(t1b, cZ, [[g, ti], [1, g]], bass),
                        op=sub)
                    # m2 = (zim_i, zre_i) bcast * (zre[i:], zim[i:])
                    t2 = tpool.tile([P, 2, N, g], f16, tag="t2")
                    t2b = t2[:, :, :, :]
                    pick(2 * ti * g, 0.53).tensor_tensor(
                        out=_ap(t2b, 0, [[cZ, 2], [g, ti], [1, g]], bass),
                        in0=_ap(z16b, cZ + i * g, [[-cZ, 2], [0, ti], [1, g]], bass),
                        in1=_ap(z16b, i * g, [[cZ, 2], [g, ti], [1, g]], bass),
                        op=mult)
                    pick(ti * g, 0.53).tensor_tensor(
                        out=_ap(w16b, cW + off * g, [[g, ti], [1, g]], bass),
                        in0=_ap(t2b, 0, [[g, ti], [1, g]], bass),
                        in1=_ap(t2b, cZ, [[g, ti], [1, g]], bass),
                        op=add)

                # ---- zz stage: zz16 [P, 2, 35, g] (R, I) ----
                zz16 = zzp.tile([P, 2, M, g], f16)
                zzb = zz16[:, :, :, :]
                zzR = 0
                zzI = cA
                for (i, j) in WPAIRS:
                    tk = N - j
                    pr = WOFF[(i, j)]
                    zo = ZOFF[(i, j)]
                    t3 = tpool.tile([P, 2, N, g], f16, tag="t1")
                    t3b = t3[:, :, :, :]
                    pick(2 * tk * g, 0.53).tensor_tensor(
                        out=_ap(t3b, 0, [[cZ, 2], [g, tk], [1, g]], bass),
                        in0=_ap(w16b, pr * g, [[cW, 2], [0, tk], [1, g]],
                                bass),
                        in1=_ap(z16b, j * g, [[cZ, 2], [g, tk], [1, g]], bass),
                        op=mult)
                    pick(tk * g, 0.53).tensor_tensor(
                        out=_ap(zzb, zzR + zo * g, [[g, tk], [1, g]], bass),
                        in0=_ap(t3b, 0, [[g, tk], [1, g]], bass),
                        in1=_ap(t3b, cZ, [[g, tk], [1, g]], bass),
                        op=sub)
                    t4 = tpool.tile([P, 2, N, g], f16, tag="t2")
                    t4b = t4[:, :, :, :]
                    pick(2 * tk * g, 0.53).tensor_tensor(
                        out=_ap(t4b, 0, [[cZ, 2], [g, tk], [1, g]], bass),
                        in0=_ap(w16b, cW + pr * g,
                                [[-cW, 2], [0, tk], [1, g]], bass),
                        in1=_ap(z16b, j * g, [[cZ, 2], [g, tk], [1, g]], bass),
                        op=mult)
                    pick(tk * g, 0.53).tensor_tensor(
                        out=_ap(zzb, zzI + zo * g, [[g, tk], [1, g]], bass),
                        in0=_ap(t4b, 0, [[g, tk], [1, g]], bass),
                        in1=_ap(t4b, cZ, [[g, tk], [1, g]], bass),
                        op=add)

                # ---- Gauss slabs: gs [P, 3, 35, g] = S, Sn, D ----
                gs = gsp.tile([P, 3, M, g], f16)
                gsb = gs[:, :, :, :]
                pick(cA, 0.53).tensor_tensor(
                    out=_ap(gsb, 0, [[1, cA]], bass),
                    in0=_ap(zzb, 0, [[1, cA]], bass),
                    in1=_ap(zzb, cA, [[1, cA]], bass), op=add)        # S = R+I
                nc.vector.tensor_scalar_mul(
                    out=_ap(gsb, cA, [[1, cA]], bass),
                    in0=_ap(gsb, 0, [[1, cA]], bass), scalar1=-1.0)   # Sn = -S
                eng["v"] += 100 + 0.27 * cA
                pick(cA, 0.53).tensor_tensor(
                    out=_ap(gsb, 2 * cA, [[1, cA]], bass),
                    in0=_ap(zzb, cA, [[1, cA]], bass),
                    in1=_ap(zzb, 0, [[1, cA]], bass), op=sub)         # D = I-R

                # ---- products: iterate p once; k-tiles feed BOTH streams ----
                # Interleaved p order (psum-heavy low-p alternating with
                # direct-route high-p) keeps PE/ACT and GPS busy concurrently
                # instead of in two serialized phases.
                # Out tile per (kind, chunk-index) tag, opened lazily, closed
                # when every block column has been written (static fill map).
                st = {}
                for kind in ("re", "im"):
                    chunks = _make_chunks(kind)
                    blkmap = {}  # p -> (ci, cb_in_chunk)
                    for ci, (_cb, _cols, bl) in enumerate(chunks):
                        for (p, cb, w) in bl:
                            blkmap[p] = (ci, cb)
                    st[kind] = {"chunks": chunks, "blk": blkmap,
                                "tile": {}, "pfx": {},
                                "left": {ci: c[1] for ci, c in
                                         enumerate(chunks)}}

                slot = None  # [ps_ap, off, segs]; seg=[ocb, cw, oc0, po, nc]

                def drain():
                    nonlocal slot
                    if slot is None:
                        return
                    pb, _soff, segs = slot
                    if "no_drain" not in DBG:
                        for (socb, cw, oc0, po, ncols) in segs:
                            nc.scalar.copy(
                                out=_ap(socb, oc0, [[cw, g], [1, ncols]], bass),
                                in_=_ap(pb, po, [[1, g], [g, ncols]], bass))
                    slot = None

                def open_chunk(kind, ci):
                    s = st[kind]
                    cols = s["chunks"][ci][1]
                    t = outp.tile([P, g, cols], f16, tag=f"oc_{kind}{ci}")
                    s["tile"][ci] = t[:, :, :]

                def dma_cols(kind, ci, c0, c1):
                    if "no_dma_out" in DBG or c1 <= c0:
                        return
                    s = st[kind]
                    colbase, cols, _bl = s["chunks"][ci]
                    dst = out_d[r0:r0 + P * g,
                                colbase + c0:colbase + c1].rearrange(
                        "(p g) f -> p g f", g=g)
                    nc.sync.dma_start(
                        out=dst,
                        in_=_ap(s["tile"][ci], c0, [[cols, g], [1, c1 - c0]],
                                bass))

                def close_chunk(kind, ci):
                    s = st[kind]
                    drain()  # open slot may reference this chunk's tile
                    cols = s["chunks"][ci][1]
                    dma_cols(kind, ci, s["pfx"].get(ci, 0), cols)
                    del s["tile"][ci]

                def emit(kind, p, seg_off, kAb, offA, kBb, offB, w):
                    nonlocal slot
                    s = st[kind]
                    ci, cb0 = s["blk"][p]
                    if ci not in s["tile"]:
                        open_chunk(kind, ci)
                    cb = cb0 + seg_off
                    ocb = s["tile"][ci]
                    cw = s["chunks"][ci][1]
                    if w <= WC:
                        pick(w * g, 0.85).tensor_tensor(
                            out=_ap(ocb, cb, [[cw, g], [1, w]], bass),
                            in0=_ap(kAb, offA, [[1, g], [g, w]], bass),
                            in1=_ap(kBb, offB, [[1, g], [g, w]], bass),
                            op=add)
                    else:
                        # block-atomic slot: no mid-block slot boundary, so
                        # every block is one ACT drain segment
                        if slot is not None and slot[1] + w * g > SLOT:
                            drain()
                        if slot is None:
                            pst = psp.tile([P, SLOT], f32, tag="ps")
                            slot = [pst[:, :], 0, []]
                        pb, soff, segs = slot
                        segs.append([ocb, cw, cb, soff, w])
                        rem = w * g
                        fA, fB = offA, offB
                        while rem > 0:
                            n = min(rem, 512 - soff % 512)
                            pso = _ap(pb, soff, [[1, n]], bass)
                            nc.tensor.matmul(
                                pso, ident, _ap(kAb, fA, [[1, n]], bass),
                                start=True, stop=False, skip_group_check=True)
                            if "single_mm" not in DBG:
                                nc.tensor.matmul(
                                    pso, ident, _ap(kBb, fB, [[1, n]], bass),
                                    start=False, stop=True,
                                    skip_group_check=True)
                            soff += n
                            fA += n
                            fB += n
                            rem -= n
                        slot[1] = soff
                        if slot[1] == SLOT:
                            drain()
                    s["left"][ci] -= w
                    left = s["left"][ci]
                    cols = s["chunks"][ci][1]
                    if left == 0:
                        close_chunk(kind, ci)
                    elif ci not in s["pfx"] and left <= TAILC and \
                            cols - left >= 256:
                        # fire the bulk of the chunk early; only a small
                        # suffix DMA remains at close (shrinks the end-of-
                        # kernel DMA tail). Drain first so psum-routed cols
                        # are in SBUF before the DMA reads them.
                        drain()
                        dma_cols(kind, ci, 0, cols - left)
                        s["pfx"][ci] = cols - left

                if INTERLEAVE:
                    lowp = [p for p in range(M) if M - p > WC]
                    highp = [p for p in range(M - 1, -1, -1) if M - p <= WC]
                    p_order = []
                    for i in range(max(len(lowp), len(highp))):
                        if i < len(lowp):
                            p_order.append(lowp[i])
                        if i < len(highp):
                            p_order.append(highp[i])
                else:
                    p_order = list(range(M))

                cK = QCAP * g
                for p in p_order:
                    q0 = p
                    while q0 < M:
                        ws = min(QCAP, M - q0)  # segment [q0, q0+ws)
                        kt = ktp.tile([P, 3, QCAP, g], f16, tag="kt")
                        ktb = kt[:, :, :, :]
                        # fused: (k1, k2, k3n) = (S[p], Rp, Ip)*(R, Sn, D)[q0:]
                        pick(3 * ws * g, 0.53).tensor_tensor(
                            out=_ap(ktb, 0, [[cK, 3], [g, ws], [1, g]], bass),
                            in0=_ap(zzgsb, SL_S * cA + p * g,
                                    [[cA, 3], [0, ws], [1, g]], bass),
                            in1=_ap(zzgsb, SL_R * cA + q0 * g,
                                    [[2 * cA, 3], [g, ws], [1, g]], bass),
                            op=mult)
                        # re = k1 + k3n over q in [q0, q0+ws)
                        emit("re", p, q0 - p, ktb, 0, ktb, 2 * cK, ws)
                        # im = k1 + k2 over q > p
                        sk = g if q0 == p else 0  # skip diagonal col
                        if ws * g - sk > 0:
                            emit("im", p, q0 + sk // g - (p + 1), ktb, sk,
                                 ktb, cK + sk, ws - sk // g)
                        q0 += ws
                for kind in ("re", "im"):
                    assert not st[kind]["tile"], (kind, st[kind]["left"])

    nc.finalize()
    return nc


_CACHED = {}


def _get_nc():
    if "nc" not in _CACHED:
        _CACHED["nc"] = build_bass()
    return _CACHED["nc"]


def kernel(z_re, z_im):
    from concourse.bass_utils import run_bass_kernel_spmd

    z_re = np.ascontiguousarray(np.asarray(z_re, dtype=np.float32))
    z_im = np.ascontiguousarray(np.asarray(z_im, dtype=np.float32))
    assert z_re.shape == (B_FULL, N), z_re.shape

    nc = _get_nc()
    ident = np.eye(P, dtype=np.float16)
    in_maps = []
    for c in range(NC):
        sl = slice(c * B_LOCAL, (c + 1) * B_LOCAL)
        in_maps.append({
            "z_re": np.ascontiguousarray(z_re[sl]),
            "z_im": np.ascontiguousarray(z_im[sl]),
            "ident": ident,
        })
    res = run_bass_kernel_spmd(nc, in_maps, core_ids=list(range(NC)))
    out = np.concatenate([res.results[c]["out"] for c in range(NC)], axis=0)
    return out.astype(np.float32)


# revision 4
# speedup vs baseline: 1.0768x; 1.0030x over previous
"""Trainium2 Bass kernel v2 for nn_Bihomogeneous_k3 (fp16 feature-major).

Math per batch row (complex z dim 5 given as z_re/z_im):
  zz[m]  = z_i z_j z_k for 35 triples i<=j<=k (lex order)
  prod   = zz[p] * conj(zz[q]) for 630 pairs p<=q
  out    = [Re(prod) (630) | Im(prod) strict p<q (595)] -> [B, 1225]

Distribution: pure data parallel over 8 cores (batch sharded).

Per-core design (B_local=16384, P=128 partitions, G=64 rows/partition/mt,
2 megatiles). All pair-product compute in fp16, FEATURE-MAJOR layout
[P, c, feat, g]: every DVE op's innermost dim is the packed g-run, which
qualifies fp16 TensorTensor for the 2x_1p DVE perf mode (0.52 ns/elem).

Gauss 3-mult complex products per p-block (q in [p,34]):
  slabs: S = R+I, Sn = -S, D = I-R   (R/I = re/im of zz)
  k1 = S[p]*R[q]; k2 = Rp*Sn[q]; k3n = Ip*D[q]
  re(p,q) = k1 + k3n  (q>=p; q=p gives the diagonal Rp^2+Ip^2 for free)
  im(p,q) = k1 + k2   (q>p)
Adds are routed either:
  - PSUM: two fp16 identity-weight matmuls (start=True then accumulate)
    into 2048-elem PSUM slots; ACT drains (and transposes to batch-major,
    converting f32->fp16) into the output chunk, or
  - direct: small tail blocks (w<=WC) as one transposing TT add on GPS/DVE.
Output chunks [P, g, ~306 cols] fp16 are DMA'd out (>=512B contiguous runs
-> full 360B/ns cost-model DMA rate); host concatenates + casts to f32.

Tuning status (timeline-sim 230819 ns/core, HW rel-err 2.11e-03): the
engine balance is DVE ~= GPSIMD ~= 184us busy (the multiply wall), ACT
~127, DMA ~115 floor. Closed/negative results, do not retry blindly:
  - GPSIMD ScalarTensorTensor (0.60-eff TensorScalarPtr pricing) is
    ISA-ILLEGAL on Pool (neuron_isa_check_opcode_on_engine rejects it);
    plain gpsimd.tensor_scalar_* IS legal (baseline-proven).
  - Fused 3-slab k-op, interleaved p-order, staged direct adds
    (packed add + transpose-copy), zz double-buffering via QCAP<=13,
    QCAP>=24/KT_BUFS=2, mid-block PSUM slot splits, greedy jitter
    (10 seeds, all 24-46us worse): every restructure that deepens dep
    chains or shrinks op granularity loses more to pipeline stalls than
    engine arithmetic predicts.
  - Greedy constants (GPS_RATE=2.3, direct-add DVE rate 0.85, WC=14,
    SPLIT_P=10, QCAP=20, KT_BUFS=3, TAILC=96) are each the argmin of
    valid sweeps; 90-config random scatter found nothing better.
  - DVE fp16 ts_mul KEEPS 2x_2p even with transposing APs (0.52/elem);
    TensorTensor only has 2x_1p (needs packed last dims everywhere).
"""
import sys

sys.path.insert(0, "/opt/trn_rl_repo")

import numpy as np

N = 5
NC = 8
B_FULL = 131072
B_LOCAL = B_FULL // NC
P = 128
G = 64
N_MT = B_LOCAL // (P * G)  # 2

# ---- index tables ----
WPAIRS = [(i, j) for i in range(N) for j in range(i, N)]  # 15 lex
WOFF = {}
_o = 0
for (i, j) in WPAIRS:
    WOFF[(i, j)] = _o
    _o += 1
ZOFF = {}
_o = 0
for (i, j) in WPAIRS:
    ZOFF[(i, j)] = _o
    _o += N - j
M = _o
assert M == 35

REOFF = np.concatenate([[0], np.cumsum([M - p for p in range(M)])]).astype(int)
IMOFF = np.concatenate([[0], np.cumsum([M - 1 - p for p in range(M - 1)])]).astype(int)
N_RE = int(REOFF[M])      # 630
N_IM = int(IMOFF[M - 1])  # 595
N_OUT = N_RE + N_IM       # 1225

WC = 14          # blocks with add-width <= WC go the direct (no-PSUM) route
SPLIT_P = 10     # stream chunk boundary: blocks p < SPLIT_P | p >= SPLIT_P
QCAP = 20        # max q-segment width (caps k-tile SBUF footprint)
DBG = set()      # timing knockouts: no_dma_out / no_drain / single_mm / no_prod
KT_BUFS = 3      # k-tile ring depth
ZZ_BUFS = 1      # zz/gs double buffering (2 = overlap next mt's zz stage)
SLOT = 2048      # psum slot size (f32 elems); SLOT * PS_BUFS <= 4096
PS_BUFS = 2
INTERLEAVE = False


def _make_chunks(kind):
    """Two column chunks for one output stream ('re' or 'im'), split at
    p == SPLIT_P. All chunk widths land in [256, 330] so fp16 DMA runs are
    >= 512B. Returns [(colbase, cols, [(p, cb_in_chunk, w), ...]), ...] with
    colbase in FULL output coordinates (im stream offset by N_RE)."""
    if kind == "re":
        blocks = [(p, int(REOFF[p]), M - p) for p in range(M)]
    else:
        blocks = [(p, N_RE + int(IMOFF[p]), M - 1 - p) for p in range(M - 1)]
    chunks = []
    cur, base, cols = [], blocks[0][1], 0
    for p, cb, w in blocks:
        if p == SPLIT_P and cur:
            chunks.append((base, cols, cur))
            base, cols, cur = cb, 0, []
        cur.append((p, cb - base, w))
        cols += w
    if cur:
        chunks.append((base, cols, cur))
    assert all(256 <= c[1] <= 330 for c in chunks), [c[1] for c in chunks]
    return chunks


def _ap(base_ap, offset_elems, dims, bassmod):
    return bassmod.AP(tensor=base_ap.tensor, offset=base_ap.offset + offset_elems,
                      ap=[list(base_ap.ap[0])] + [list(d) for d in dims])


def build_bass(g=G, n_mt=N_MT):
    import concourse.bacc as bacc
    import concourse.bass as bass
    import concourse.tile as tile
    from concourse import mybir
    from contextlib import ExitStack

    f32 = mybir.dt.float32
    f16 = mybir.dt.float16
    b_local = P * g * n_mt

    nc = bacc.Bacc(None)
    z_re_d = nc.dram_tensor("z_re", [b_local, N], f32, kind="ExternalInput")
    z_im_d = nc.dram_tensor("z_im", [b_local, N], f32, kind="ExternalInput")
    ident_d = nc.dram_tensor("ident", [P, P], f16, kind="ExternalInput")
    out_d = nc.dram_tensor("out", [b_local, N_OUT], f16, kind="ExternalOutput")

    mult = mybir.AluOpType.mult
    add = mybir.AluOpType.add
    sub = mybir.AluOpType.subtract

    # greedy DVE/GPS balance (ns; fd = free elems). DVE fp16 2x_1p; GPS eff.
    eng = {"v": 0.0, "g": 0.0}

    def pick(fd, dve_rate):
        cv = eng["v"] + 100 + dve_rate * fd
        cg = eng["g"] + 131 + 2.3 * fd
        if cv <= cg:
            eng["v"] = cv
            return nc.vector
        eng["g"] = cg
        return nc.gpsimd

    with tile.TileContext(nc) as tc:
        with ExitStack() as ctx:
            const_pool = ctx.enter_context(tc.tile_pool(name="const", bufs=1))
            z32p = ctx.enter_context(tc.tile_pool(name="z32p", bufs=1))
            z16p = ctx.enter_context(tc.tile_pool(name="z16p", bufs=1))
            zzp = ctx.enter_context(tc.tile_pool(name="zzp", bufs=ZZ_BUFS))
            wpool = ctx.enter_context(tc.tile_pool(name="wp", bufs=1))
            gsp = ctx.enter_context(tc.tile_pool(name="gsp", bufs=ZZ_BUFS))
            tpool = ctx.enter_context(tc.tile_pool(name="tp", bufs=1))
            ktp = ctx.enter_context(tc.tile_pool(name="ktp", bufs=KT_BUFS))
            outp = ctx.enter_context(tc.tile_pool(name="outp", bufs=1))
            psp = ctx.enter_context(tc.tile_pool(name="psp", bufs=PS_BUFS,
                                                 space="PSUM"))

            ident = const_pool.tile([P, P], f16)
            nc.sync.dma_start(out=ident, in_=ident_d[:, :])
            # Warm-up matmul (weight load); uses the regular psum slot ring so
            # PSUM stays at exactly 2 x 2048 f32 = 16KB/partition.
            warm = psp.tile([P, SLOT], f32, tag="ps")
            nc.tensor.matmul(warm[:, 0:1], ident, ident[:, 0:1], start=True,
                             stop=True)

            for mt in range(n_mt):
                r0 = mt * P * g

                # ---- load z batch-major, convert+transpose to feature-major
                z32 = z32p.tile([P, 2, g, N], f32)
                nc.sync.dma_start(
                    out=z32[:, 0, :, :],
                    in_=z_re_d[r0:r0 + P * g, :].rearrange("(p g) f -> p g f", g=g))
                nc.sync.dma_start(
                    out=z32[:, 1, :, :],
                    in_=z_im_d[r0:r0 + P * g, :].rearrange("(p g) f -> p g f", g=g))
                z16 = z16p.tile([P, 2, N, g], f16)
                z32b = z32[:, :, :, :]
                z16b = z16[:, :, :, :]
                # iterate (c, i, gi): in (c, gi, i) strides, out packed
                nc.vector.tensor_scalar_mul(
                    out=_ap(z16b, 0, [[N * g, 2], [g, N], [1, g]], bass),
                    in0=_ap(z32b, 0, [[N * g, 2], [1, N], [N, g]], bass),
                    scalar1=1.0)
                eng["v"] += 100 + 1.042 * 2 * N * g
                cZ = N * g

                # ---- w stage: w16 [P, 2, 15, g] ----
                w16 = wpool.tile([P, 2, len(WPAIRS), g], f16)
                w16b = w16[:, :, :, :]
                wbase = 0
                cA = M * g
                cW = len(WPAIRS) * g
                for i in range(N):
                    ti = N - i
                    off = WOFF[(i, i)]
                    t1 = tpool.tile([P, 2, N, g], f16, tag="t1")
                    t1b = t1[:, :, :, :]
                    # m1 = (zre_i, zim_i) bcast * (zre[i:], zim[i:])
                    pick(2 * ti * g, 0.53).tensor_tensor(
                        out=_ap(t1b, 0, [[cZ, 2], [g, ti], [1, g]], bass),
                        in0=_ap(z16b, i * g, [[cZ, 2], [0, ti], [1, g]], bass),
                        in1=_ap(z16b, i * g, [[cZ, 2], [g, ti], [1, g]], bass),
                        op=mult)
                    pick(ti * g, 0.53).tensor_tensor(
                        out=_ap(w16b, wbase + off * g, [[g, ti], [1, g]], bass),
                        in0=_ap(t1b, 0, [[g, ti], [1, g]], bass),
                        in1=_ap(t1b, cZ, [[g, ti], [1, g]], bass),
                        op=sub)
                    # m2 = (zim_i, zre_i) bcast * (zre[i:], zim[i:])
                    t2 = tpool.tile([P, 2, N, g], f16, tag="t2")
                    t2b = t2[:, :, :, :]
                    pick(2 * ti * g, 0.53).tensor_tensor(
                        out=_ap(t2b, 0, [[cZ, 2], [g, ti], [1, g]], bass),
                        in0=_ap(z16b, cZ + i * g, [[-cZ, 2], [0, ti], [1, g]], bass),
                        in1=_ap(z16b, i * g, [[cZ, 2], [g, ti], [1, g]], bass),
                        op=mult)
                    pick(ti * g, 0.53).tensor_tensor(
                        out=_ap(w16b, cW + off * g, [[g, ti], [1, g]], bass),
                        in0=_ap(t2b, 0, [[g, ti], [1, g]], bass),
                        in1=_ap(t2b, cZ, [[g, ti], [1, g]], bass),
                        op=add)

                # ---- zz stage: zz16 [P, 2, 35, g] (R, I) ----
                zz16 = zzp.tile([P, 2, M, g], f16)
                zzb = zz16[:, :, :, :]
                zzR = 0
                zzI = cA
                for (i, j) in WPAIRS:
                    tk = N - j
                    pr = WOFF[(i, j)]
                    zo = ZOFF[(i, j)]
                    t3 = tpool.tile([P, 2, N, g], f16, tag="t1")
                    t3b = t3[:, :, :, :]
                    pick(2 * tk * g, 0.53).tensor_tensor(
                        out=_ap(t3b, 0, [[cZ, 2], [g, tk], [1, g]], bass),
                        in0=_ap(w16b, pr * g, [[cW, 2], [0, tk], [1, g]],
                                bass),
                        in1=_ap(z16b, j * g, [[cZ, 2], [g, tk], [1, g]], bass),
                        op=mult)
                    pick(tk * g, 0.53).tensor_tensor(
                        out=_ap(zzb, zzR + zo * g, [[g, tk], [1, g]], bass),
                        in0=_ap(t3b, 0, [[g, tk], [1, g]], bass),
                        in1=_ap(t3b, cZ, [[g, tk], [1, g]], bass),
                        op=sub)
                    t4 = tpool.tile([P, 2, N, g], f16, tag="t2")
                    t4b = t4[:, :, :, :]
                    pick(2 * tk * g, 0.53).tensor_tensor(
                        out=_ap(t4b, 0, [[cZ, 2], [g, tk], [1, g]], bass),
                        in0=_ap(w16b, cW + pr * g,
                                [[-cW, 2], [0, tk], [1, g]], bass),
                        in1=_ap(z16b, j * g, [[cZ, 2], [g, tk], [1, g]], bass),
                        op=mult)
                    pick(tk * g, 0.53).tensor_tensor(
                        out=_ap(zzb, zzI + zo * g, [[g, tk], [1, g]], bass),
                        in0=_ap(t4b, 0, [[g, tk], [1, g]], bass),
                        in1=_ap(t4b, cZ, [[g, tk], [1, g]], bass),
                        op=add)

                # ---- Gauss slabs: gs [P, 3, 35, g] = S, Sn, D ----
                gs = gsp.tile([P, 3, M, g], f16)
                gsb = gs[:, :, :, :]
                pick(cA, 0.53).tensor_tensor(
                    out=_ap(gsb, 0, [[1, cA]], bass),
                    in0=_ap(zzb, 0, [[1, cA]], bass),
                    in1=_ap(zzb, cA, [[1, cA]], bass), op=add)        # S = R+I
                nc.vector.tensor_scalar_mul(
                    out=_ap(gsb, cA, [[1, cA]], bass),
                    in0=_ap(gsb, 0, [[1, cA]], bass), scalar1=-1.0)   # Sn = -S
                eng["v"] += 100 + 0.27 * cA
                pick(cA, 0.53).tensor_tensor(
                    out=_ap(gsb, 2 * cA, [[1, cA]], bass),
                    in0=_ap(zzb, cA, [[1, cA]], bass),
                    in1=_ap(zzb, 0, [[1, cA]], bass), op=sub)         # D = I-R

                # ---- products: iterate p once; k-tiles feed BOTH streams ----
                # Interleaved p order (psum-heavy low-p alternating with
                # direct-route high-p) keeps PE/ACT and GPS busy concurrently
                # instead of in two serialized phases.
                # Out tile per (kind, chunk-index) tag, opened lazily, closed
                # when every block column has been written (static fill map).
                st = {}
                for kind in ("re", "im"):
                    chunks = _make_chunks(kind)
                    blkmap = {}  # p -> (ci, cb_in_chunk)
                    for ci, (_cb, _cols, bl) in enumerate(chunks):
                        for (p, cb, w) in bl:
                            blkmap[p] = (ci, cb)
                    st[kind] = {"chunks": chunks, "blk": blkmap,
                                "tile": {}, "pfx": {},
                                "left": {ci: c[1] for ci, c in
                                         enumerate(chunks)}}

                slot = None  # [ps_ap, off, segs]; seg=[ocb, cw, oc0, po, nc]

                def drain():
                    nonlocal slot
                    if slot is None:
                        return
                    pb, _soff, segs = slot
                    if "no_drain" not in DBG:
                        for (socb, cw, oc0, po, ncols) in segs:
                            nc.scalar.copy(
                                out=_ap(socb, oc0, [[cw, g], [1, ncols]], bass),
                                in_=_ap(pb, po, [[1, g], [g, ncols]], bass))
                    slot = None

                def open_chunk(kind, ci):
                    s = st[kind]
                    cols = s["chunks"][ci][1]
                    t = outp.tile([P, g, cols], f16, tag=f"oc_{kind}{ci}")
                    s["tile"][ci] = t[:, :, :]

                def dma_cols(kind, ci, c0, c1):
                    if "no_dma_out" in DBG or c1 <= c0:
                        return
                    s = st[kind]
                    colbase, cols, _bl = s["chunks"][ci]
                    dst = out_d[r0:r0 + P * g,
                                colbase + c0:colbase + c1].rearrange(
                        "(p g) f -> p g f", g=g)
                    nc.sync.dma_start(
                        out=dst,
                        in_=_ap(s["tile"][ci], c0, [[cols, g], [1, c1 - c0]],
                                bass))

                def close_chunk(kind, ci):
                    s = st[kind]
                    drain()  # open slot may reference this chunk's tile
                    cols = s["chunks"][ci][1]
                    dma_cols(kind, ci, s["pfx"].get(ci, 0), cols)
                    del s["tile"][ci]

                def emit(kind, p, seg_off, kAb, offA, kBb, offB, w):
                    nonlocal slot
                    s = st[kind]
                    ci, cb0 = s["blk"][p]
                    if ci not in s["tile"]:
                        open_chunk(kind, ci)
                    cb = cb0 + seg_off
                    ocb = s["tile"][ci]
                    cw = s["chunks"][ci][1]
                    if w <= WC:
                        pick(w * g, 0.85).tensor_tensor(
                            out=_ap(ocb, cb, [[cw, g], [1, w]], bass),
                            in0=_ap(kAb, offA, [[1, g], [g, w]], bass),
                            in1=_ap(kBb, offB, [[1, g], [g, w]], bass),
                            op=add)
                    else:
                        # block-atomic slot: no mid-block slot boundary, so
                        # every block is one ACT drain segment
                        if slot is not None and slot[1] + w * g > SLOT:
                            drain()
                        if slot is None:
                            pst = psp.tile([P, SLOT], f32, tag="ps")
                            slot = [pst[:, :], 0, []]
                        pb, soff, segs = slot
                        segs.append([ocb, cw, cb, soff, w])
                        rem = w * g
                        fA, fB = offA, offB
                        while rem > 0:
                            n = min(rem, 512 - soff % 512)
                            pso = _ap(pb, soff, [[1, n]], bass)
                            nc.tensor.matmul(
                                pso, ident, _ap(kAb, fA, [[1, n]], bass),
                                start=True, stop=False, skip_group_check=True)
                            if "single_mm" not in DBG:
                                nc.tensor.matmul(
                                    pso, ident, _ap(kBb, fB, [[1, n]], bass),
                                    start=False, stop=True,
                                    skip_group_check=True)
                            soff += n
                            fA += n
                            fB += n
                            rem -= n
                        slot[1] = soff
                        if slot[1] == SLOT:
                            drain()
                    s["left"][ci] -= w
                    left = s["left"][ci]
                    cols = s["chunks"][ci][1]
                    if left == 0:
                        close_chunk(kind, ci)
                    elif ci not in s["pfx"] and left <= TAILC and \
                            cols - left >= 256:
                        # fire the bulk of the chunk early; only a small
                        # suffix DMA remains at close (shrinks the end-of-
                        # kernel DMA tail). Drain first so psum-routed cols
                        # are in SBUF before the DMA reads them.
                        drain()
                        dma_cols(kind, ci, 0, cols - left)
                        s["pfx"][ci] = cols - left

                if INTERLEAVE:
                    lowp = [p for p in range(M) if M - p > WC]
                    highp = [p for p in range(M - 1, -1, -1) if M - p <= WC]
                    p_order = []
                    for i in range(max(len(lowp), len(highp))):
                        if i < len(lowp):
                            p_order.append(lowp[i])
                        if i < len(highp):
                            p_order.append(highp[i])
                else:
                    p_order = list(range(M))

                cK = QCAP * g
                for p in p_order:
                    q0 = p
                    while q0 < M:
                        ws = min(QCAP, M - q0)  # segment [q0, q0+ws)
                        kt = ktp.tile([P, 3, QCAP, g], f16, tag="kt")
                        ktb = kt[:, :, :, :]
                        # fused: (k1, k2, k3n) = (S[p], Rp, Ip)*(R, Sn, D)[q0:]
                        pick(3 * ws * g, 0.53).tensor_tensor(
                            out=_ap(ktb, 0, [[cK, 3], [g, ws], [1, g]], bass),
                            in0=_ap(zzgsb, SL_S * cA + p * g,
                                    [[cA, 3], [0, ws], [1, g]], bass),
                            in1=_ap(zzgsb, SL_R * cA + q0 * g,
                                    [[2 * cA, 3], [g, ws], [1, g]], bass),
                            op=mult)
                        # re = k1 + k3n over q in [q0, q0+ws)
                        emit("re", p, q0 - p, ktb, 0, ktb, 2 * cK, ws)
                        # im = k1 + k2 over q > p
                        sk = g if q0 == p else 0  # skip diagonal col
                        if ws * g - sk > 0:
                            emit("im", p, q0 + sk // g - (p + 1), ktb, sk,
                                 ktb, cK + sk, ws - sk // g)
                        q0 += ws
                for kind in ("re", "im"):
                    assert not st[kind]["tile"], (kind, st[kind]["left"])

    nc.finalize()
    return nc


_CACHED = {}


def _get_nc():
    if "nc" not in _CACHED:
        _CACHED["nc"] = build_bass()
    return _CACHED["nc"]


def kernel(z_re, z_im):
    from concourse.bass_utils import run_bass_kernel_spmd

    z_re = np.ascontiguousarray(np.asarray(z_re, dtype=np.float32))
    z_im = np.ascontiguousarray(np.asarray(z_im, dtype=np.float32))
    assert z_re.shape == (B_FULL, N), z_re.shape

    nc = _get_nc()
    ident = np.eye(P, dtype=np.float16)
    in_maps = []
    for c in range(NC):
        sl = slice(c * B_LOCAL, (c + 1) * B_LOCAL)
        in_maps.append({
            "z_re": np.ascontiguousarray(z_re[sl]),
            "z_im": np.ascontiguousarray(z_im[sl]),
            "ident": ident,
        })
    res = run_bass_kernel_spmd(nc, in_maps, core_ids=list(range(NC)))
    out = np.concatenate([res.results[c]["out"] for c in range(NC)], axis=0)
    return out.astype(np.float32)


# revision 5
# speedup vs baseline: 1.0803x; 1.0032x over previous
"""Trainium2 Bass kernel v2 for nn_Bihomogeneous_k3 (fp16 feature-major).

Math per batch row (complex z dim 5 given as z_re/z_im):
  zz[m]  = z_i z_j z_k for 35 triples i<=j<=k (lex order)
  prod   = zz[p] * conj(zz[q]) for 630 pairs p<=q
  out    = [Re(prod) (630) | Im(prod) strict p<q (595)] -> [B, 1225]

Distribution: pure data parallel over 8 cores (batch sharded).

Per-core design (B_local=16384, P=128 partitions, G=64 rows/partition/mt,
2 megatiles). All pair-product compute in fp16, FEATURE-MAJOR layout
[P, c, feat, g]: every DVE op's innermost dim is the packed g-run, which
qualifies fp16 TensorTensor for the 2x_1p DVE perf mode (0.52 ns/elem).

Gauss 3-mult complex products per p-block (q in [p,34]):
  slabs: S = R+I, Sn = -S, D = I-R   (R/I = re/im of zz)
  k1 = S[p]*R[q]; k2 = Rp*Sn[q]; k3n = Ip*D[q]
  re(p,q) = k1 + k3n  (q>=p; q=p gives the diagonal Rp^2+Ip^2 for free)
  im(p,q) = k1 + k2   (q>p)
Adds are routed either:
  - PSUM: two fp16 identity-weight matmuls (start=True then accumulate)
    into 2048-elem PSUM slots; ACT drains (and transposes to batch-major,
    converting f32->fp16) into the output chunk, or
  - direct: small tail blocks (w<=WC) as one transposing TT add on GPS/DVE.
Output chunks [P, g, ~306 cols] fp16 are DMA'd out (>=512B contiguous runs
-> full 360B/ns cost-model DMA rate); host concatenates + casts to f32.

Tuning status (timeline-sim 230132 ns/core, HW rel-err 2.11e-03): the
engine balance is DVE ~= GPSIMD ~= 184us busy (the multiply wall), ACT
~127, DMA ~115 floor. Closed/negative results, do not retry blindly:
  - GPSIMD ScalarTensorTensor (0.60-eff TensorScalarPtr pricing) is
    ISA-ILLEGAL on Pool (neuron_isa_check_opcode_on_engine rejects it);
    plain gpsimd.tensor_scalar_* IS legal (baseline-proven).
  - Fused 3-slab k-op, interleaved p-order, staged direct adds
    (packed add + transpose-copy), zz double-buffering via QCAP<=13,
    QCAP>=24/KT_BUFS=2, mid-block PSUM slot splits, greedy jitter
    (10 seeds, all 24-46us worse): every restructure that deepens dep
    chains or shrinks op granularity loses more to pipeline stalls than
    engine arithmetic predicts.
  - Greedy constants (GPS_RATE=2.314, direct-add DVE rate 0.811,
    mult rate 0.5351, WC=14, SPLIT_P=10, QCAP=20, KT_BUFS=3, TAILC=128)
    found by a tight joint scatter around the swept per-axis argmins.
  - DVE fp16 ts_mul KEEPS 2x_2p even with transposing APs (0.52/elem);
    TensorTensor only has 2x_1p (needs packed last dims everywhere).
"""
import sys

sys.path.insert(0, "/opt/trn_rl_repo")

import numpy as np

N = 5
NC = 8
B_FULL = 131072
B_LOCAL = B_FULL // NC
P = 128
G = 64
N_MT = B_LOCAL // (P * G)  # 2

# ---- index tables ----
WPAIRS = [(i, j) for i in range(N) for j in range(i, N)]  # 15 lex
WOFF = {}
_o = 0
for (i, j) in WPAIRS:
    WOFF[(i, j)] = _o
    _o += 1
ZOFF = {}
_o = 0
for (i, j) in WPAIRS:
    ZOFF[(i, j)] = _o
    _o += N - j
M = _o
assert M == 35

REOFF = np.concatenate([[0], np.cumsum([M - p for p in range(M)])]).astype(int)
IMOFF = np.concatenate([[0], np.cumsum([M - 1 - p for p in range(M - 1)])]).astype(int)
N_RE = int(REOFF[M])      # 630
N_IM = int(IMOFF[M - 1])  # 595
N_OUT = N_RE + N_IM       # 1225

WC = 14          # blocks with add-width <= WC go the direct (no-PSUM) route
SPLIT_P = 10     # stream chunk boundary: blocks p < SPLIT_P | p >= SPLIT_P
QCAP = 20        # max q-segment width (caps k-tile SBUF footprint)
DBG = set()      # timing knockouts: no_dma_out / no_drain / single_mm / no_prod
KT_BUFS = 3      # k-tile ring depth
ZZ_BUFS = 1      # zz/gs double buffering (2 = overlap next mt's zz stage)
SLOT = 2048      # psum slot size (f32 elems); SLOT * PS_BUFS <= 4096
PS_BUFS = 2
INTERLEAVE = False


def _make_chunks(kind):
    """Two column chunks for one output stream ('re' or 'im'), split at
    p == SPLIT_P. All chunk widths land in [256, 330] so fp16 DMA runs are
    >= 512B. Returns [(colbase, cols, [(p, cb_in_chunk, w), ...]), ...] with
    colbase in FULL output coordinates (im stream offset by N_RE)."""
    if kind == "re":
        blocks = [(p, int(REOFF[p]), M - p) for p in range(M)]
    else:
        blocks = [(p, N_RE + int(IMOFF[p]), M - 1 - p) for p in range(M - 1)]
    chunks = []
    cur, base, cols = [], blocks[0][1], 0
    for p, cb, w in blocks:
        if p == SPLIT_P and cur:
            chunks.append((base, cols, cur))
            base, cols, cur = cb, 0, []
        cur.append((p, cb - base, w))
        cols += w
    if cur:
        chunks.append((base, cols, cur))
    assert all(256 <= c[1] <= 330 for c in chunks), [c[1] for c in chunks]
    return chunks


def _ap(base_ap, offset_elems, dims, bassmod):
    return bassmod.AP(tensor=base_ap.tensor, offset=base_ap.offset + offset_elems,
                      ap=[list(base_ap.ap[0])] + [list(d) for d in dims])


def build_bass(g=G, n_mt=N_MT):
    import concourse.bacc as bacc
    import concourse.bass as bass
    import concourse.tile as tile
    from concourse import mybir
    from contextlib import ExitStack

    f32 = mybir.dt.float32
    f16 = mybir.dt.float16
    b_local = P * g * n_mt

    nc = bacc.Bacc(None)
    z_re_d = nc.dram_tensor("z_re", [b_local, N], f32, kind="ExternalInput")
    z_im_d = nc.dram_tensor("z_im", [b_local, N], f32, kind="ExternalInput")
    ident_d = nc.dram_tensor("ident", [P, P], f16, kind="ExternalInput")
    out_d = nc.dram_tensor("out", [b_local, N_OUT], f16, kind="ExternalOutput")

    mult = mybir.AluOpType.mult
    add = mybir.AluOpType.add
    sub = mybir.AluOpType.subtract

    # greedy DVE/GPS balance (ns; fd = free elems). DVE fp16 2x_1p; GPS eff.
    eng = {"v": 0.0, "g": 0.0}

    def pick(fd, dve_rate):
        cv = eng["v"] + 100 + dve_rate * fd
        cg = eng["g"] + 131 + 2.3 * fd
        if cv <= cg:
            eng["v"] = cv
            return nc.vector
        eng["g"] = cg
        return nc.gpsimd

    with tile.TileContext(nc) as tc:
        with ExitStack() as ctx:
            const_pool = ctx.enter_context(tc.tile_pool(name="const", bufs=1))
            z32p = ctx.enter_context(tc.tile_pool(name="z32p", bufs=1))
            z16p = ctx.enter_context(tc.tile_pool(name="z16p", bufs=1))
            zzp = ctx.enter_context(tc.tile_pool(name="zzp", bufs=ZZ_BUFS))
            wpool = ctx.enter_context(tc.tile_pool(name="wp", bufs=1))
            gsp = ctx.enter_context(tc.tile_pool(name="gsp", bufs=ZZ_BUFS))
            tpool = ctx.enter_context(tc.tile_pool(name="tp", bufs=1))
            ktp = ctx.enter_context(tc.tile_pool(name="ktp", bufs=KT_BUFS))
            outp = ctx.enter_context(tc.tile_pool(name="outp", bufs=1))
            psp = ctx.enter_context(tc.tile_pool(name="psp", bufs=PS_BUFS,
                                                 space="PSUM"))

            ident = const_pool.tile([P, P], f16)
            nc.sync.dma_start(out=ident, in_=ident_d[:, :])
            # Warm-up matmul (weight load); uses the regular psum slot ring so
            # PSUM stays at exactly 2 x 2048 f32 = 16KB/partition.
            warm = psp.tile([P, SLOT], f32, tag="ps")
            nc.tensor.matmul(warm[:, 0:1], ident, ident[:, 0:1], start=True,
                             stop=True)

            for mt in range(n_mt):
                r0 = mt * P * g

                # ---- load z batch-major, convert+transpose to feature-major
                z32 = z32p.tile([P, 2, g, N], f32)
                nc.sync.dma_start(
                    out=z32[:, 0, :, :],
                    in_=z_re_d[r0:r0 + P * g, :].rearrange("(p g) f -> p g f", g=g))
                nc.sync.dma_start(
                    out=z32[:, 1, :, :],
                    in_=z_im_d[r0:r0 + P * g, :].rearrange("(p g) f -> p g f", g=g))
                z16 = z16p.tile([P, 2, N, g], f16)
                z32b = z32[:, :, :, :]
                z16b = z16[:, :, :, :]
                # iterate (c, i, gi): in (c, gi, i) strides, out packed
                nc.vector.tensor_scalar_mul(
                    out=_ap(z16b, 0, [[N * g, 2], [g, N], [1, g]], bass),
                    in0=_ap(z32b, 0, [[N * g, 2], [1, N], [N, g]], bass),
                    scalar1=1.0)
                eng["v"] += 100 + 1.042 * 2 * N * g
                cZ = N * g

                # ---- w stage: w16 [P, 2, 15, g] ----
                w16 = wpool.tile([P, 2, len(WPAIRS), g], f16)
                w16b = w16[:, :, :, :]
                wbase = 0
                cA = M * g
                cW = len(WPAIRS) * g
                for i in range(N):
                    ti = N - i
                    off = WOFF[(i, i)]
                    t1 = tpool.tile([P, 2, N, g], f16, tag="t1")
                    t1b = t1[:, :, :, :]
                    # m1 = (zre_i, zim_i) bcast * (zre[i:], zim[i:])
                    pick(2 * ti * g, 0.53).tensor_tensor(
                        out=_ap(t1b, 0, [[cZ, 2], [g, ti], [1, g]], bass),
                        in0=_ap(z16b, i * g, [[cZ, 2], [0, ti], [1, g]], bass),
                        in1=_ap(z16b, i * g, [[cZ, 2], [g, ti], [1, g]], bass),
                        op=mult)
                    pick(ti * g, 0.53).tensor_tensor(
                        out=_ap(w16b, wbase + off * g, [[g, ti], [1, g]], bass),
                        in0=_ap(t1b, 0, [[g, ti], [1, g]], bass),
                        in1=_ap(t1b, cZ, [[g, ti], [1, g]], bass),
                        op=sub)
                    # m2 = (zim_i, zre_i) bcast * (zre[i:], zim[i:])
                    t2 = tpool.tile([P, 2, N, g], f16, tag="t2")
                    t2b = t2[:, :, :, :]
                    pick(2 * ti * g, 0.53).tensor_tensor(
                        out=_ap(t2b, 0, [[cZ, 2], [g, ti], [1, g]], bass),
                        in0=_ap(z16b, cZ + i * g, [[-cZ, 2], [0, ti], [1, g]], bass),
                        in1=_ap(z16b, i * g, [[cZ, 2], [g, ti], [1, g]], bass),
                        op=mult)
                    pick(ti * g, 0.53).tensor_tensor(
                        out=_ap(w16b, cW + off * g, [[g, ti], [1, g]], bass),
                        in0=_ap(t2b, 0, [[g, ti], [1, g]], bass),
                        in1=_ap(t2b, cZ, [[g, ti], [1, g]], bass),
                        op=add)

                # ---- zz stage: zz16 [P, 2, 35, g] (R, I) ----
                zz16 = zzp.tile([P, 2, M, g], f16)
                zzb = zz16[:, :, :, :]
                zzR = 0
                zzI = cA
                for (i, j) in WPAIRS:
                    tk = N - j
                    pr = WOFF[(i, j)]
                    zo = ZOFF[(i, j)]
                    t3 = tpool.tile([P, 2, N, g], f16, tag="t1")
                    t3b = t3[:, :, :, :]
                    pick(2 * tk * g, 0.53).tensor_tensor(
                        out=_ap(t3b, 0, [[cZ, 2], [g, tk], [1, g]], bass),
                        in0=_ap(w16b, pr * g, [[cW, 2], [0, tk], [1, g]],
                                bass),
                        in1=_ap(z16b, j * g, [[cZ, 2], [g, tk], [1, g]], bass),
                        op=mult)
                    pick(tk * g, 0.53).tensor_tensor(
                        out=_ap(zzb, zzR + zo * g, [[g, tk], [1, g]], bass),
                        in0=_ap(t3b, 0, [[g, tk], [1, g]], bass),
                        in1=_ap(t3b, cZ, [[g, tk], [1, g]], bass),
                        op=sub)
                    t4 = tpool.tile([P, 2, N, g], f16, tag="t2")
                    t4b = t4[:, :, :, :]
                    pick(2 * tk * g, 0.53).tensor_tensor(
                        out=_ap(t4b, 0, [[cZ, 2], [g, tk], [1, g]], bass),
                        in0=_ap(w16b, cW + pr * g,
                                [[-cW, 2], [0, tk], [1, g]], bass),
                        in1=_ap(z16b, j * g, [[cZ, 2], [g, tk], [1, g]], bass),
                        op=mult)
                    pick(tk * g, 0.53).tensor_tensor(
                        out=_ap(zzb, zzI + zo * g, [[g, tk], [1, g]], bass),
                        in0=_ap(t4b, 0, [[g, tk], [1, g]], bass),
                        in1=_ap(t4b, cZ, [[g, tk], [1, g]], bass),
                        op=add)

                # ---- Gauss slabs: gs [P, 3, 35, g] = S, Sn, D ----
                gs = gsp.tile([P, 3, M, g], f16)
                gsb = gs[:, :, :, :]
                pick(cA, 0.53).tensor_tensor(
                    out=_ap(gsb, 0, [[1, cA]], bass),
                    in0=_ap(zzb, 0, [[1, cA]], bass),
                    in1=_ap(zzb, cA, [[1, cA]], bass), op=add)        # S = R+I
                nc.vector.tensor_scalar_mul(
                    out=_ap(gsb, cA, [[1, cA]], bass),
                    in0=_ap(gsb, 0, [[1, cA]], bass), scalar1=-1.0)   # Sn = -S
                eng["v"] += 100 + 0.27 * cA
                pick(cA, 0.53).tensor_tensor(
                    out=_ap(gsb, 2 * cA, [[1, cA]], bass),
                    in0=_ap(zzb, cA, [[1, cA]], bass),
                    in1=_ap(zzb, 0, [[1, cA]], bass), op=sub)         # D = I-R

                # ---- products: iterate p once; k-tiles feed BOTH streams ----
                # Interleaved p order (psum-heavy low-p alternating with
                # direct-route high-p) keeps PE/ACT and GPS busy concurrently
                # instead of in two serialized phases.
                # Out tile per (kind, chunk-index) tag, opened lazily, closed
                # when every block column has been written (static fill map).
                st = {}
                for kind in ("re", "im"):
                    chunks = _make_chunks(kind)
                    blkmap = {}  # p -> (ci, cb_in_chunk)
                    for ci, (_cb, _cols, bl) in enumerate(chunks):
                        for (p, cb, w) in bl:
                            blkmap[p] = (ci, cb)
                    st[kind] = {"chunks": chunks, "blk": blkmap,
                                "tile": {}, "pfx": {},
                                "left": {ci: c[1] for ci, c in
                                         enumerate(chunks)}}

                slot = None  # [ps_ap, off, segs]; seg=[ocb, cw, oc0, po, nc]

                def drain():
                    nonlocal slot
                    if slot is None:
                        return
                    pb, _soff, segs = slot
                    if "no_drain" not in DBG:
                        for (socb, cw, oc0, po, ncols) in segs:
                            nc.scalar.copy(
                                out=_ap(socb, oc0, [[cw, g], [1, ncols]], bass),
                                in_=_ap(pb, po, [[1, g], [g, ncols]], bass))
                    slot = None

                def open_chunk(kind, ci):
                    s = st[kind]
                    cols = s["chunks"][ci][1]
                    t = outp.tile([P, g, cols], f16, tag=f"oc_{kind}{ci}")
                    s["tile"][ci] = t[:, :, :]

                def dma_cols(kind, ci, c0, c1):
                    if "no_dma_out" in DBG or c1 <= c0:
                        return
                    s = st[kind]
                    colbase, cols, _bl = s["chunks"][ci]
                    dst = out_d[r0:r0 + P * g,
                                colbase + c0:colbase + c1].rearrange(
                        "(p g) f -> p g f", g=g)
                    nc.sync.dma_start(
                        out=dst,
                        in_=_ap(s["tile"][ci], c0, [[cols, g], [1, c1 - c0]],
                                bass))

                def close_chunk(kind, ci):
                    s = st[kind]
                    drain()  # open slot may reference this chunk's tile
                    cols = s["chunks"][ci][1]
                    dma_cols(kind, ci, s["pfx"].get(ci, 0), cols)
                    del s["tile"][ci]

                def emit(kind, p, seg_off, kAb, offA, kBb, offB, w):
                    nonlocal slot
                    s = st[kind]
                    ci, cb0 = s["blk"][p]
                    if ci not in s["tile"]:
                        open_chunk(kind, ci)
                    cb = cb0 + seg_off
                    ocb = s["tile"][ci]
                    cw = s["chunks"][ci][1]
                    if w <= WC:
                        pick(w * g, 0.85).tensor_tensor(
                            out=_ap(ocb, cb, [[cw, g], [1, w]], bass),
                            in0=_ap(kAb, offA, [[1, g], [g, w]], bass),
                            in1=_ap(kBb, offB, [[1, g], [g, w]], bass),
                            op=add)
                    else:
                        # block-atomic slot: no mid-block slot boundary, so
                        # every block is one ACT drain segment
                        if slot is not None and slot[1] + w * g > SLOT:
                            drain()
                        if slot is None:
                            pst = psp.tile([P, SLOT], f32, tag="ps")
                            slot = [pst[:, :], 0, []]
                        pb, soff, segs = slot
                        segs.append([ocb, cw, cb, soff, w])
                        rem = w * g
                        fA, fB = offA, offB
                        while rem > 0:
                            n = min(rem, 512 - soff % 512)
                            pso = _ap(pb, soff, [[1, n]], bass)
                            nc.tensor.matmul(
                                pso, ident, _ap(kAb, fA, [[1, n]], bass),
                                start=True, stop=False, skip_group_check=True)
                            if "single_mm" not in DBG:
                                nc.tensor.matmul(
                                    pso, ident, _ap(kBb, fB, [[1, n]], bass),
                                    start=False, stop=True,
                                    skip_group_check=True)
                            soff += n
                            fA += n
                            fB += n
                            rem -= n
                        slot[1] = soff
                        if slot[1] == SLOT:
                            drain()
                    s["left"][ci] -= w
                    left = s["left"][ci]
                    cols = s["chunks"][ci][1]
                    if left == 0:
                        close_chunk(kind, ci)
                    elif ci not in s["pfx"] and left <= TAILC and \
                            cols - left >= 256:
                        # fire the bulk of the chunk early; only a small
                        # suffix DMA remains at close (shrinks the end-of-
                        # kernel DMA tail). Drain first so psum-routed cols
                        # are in SBUF before the DMA reads them.
                        drain()
                        dma_cols(kind, ci, 0, cols - left)
                        s["pfx"][ci] = cols - left

                if INTERLEAVE:
                    lowp = [p for p in range(M) if M - p > WC]
                    highp = [p for p in range(M - 1, -1, -1) if M - p <= WC]
                    p_order = []
                    for i in range(max(len(lowp), len(highp))):
                        if i < len(lowp):
                            p_order.append(lowp[i])
                        if i < len(highp):
                            p_order.append(highp[i])
                else:
                    p_order = list(range(M))

                cK = QCAP * g
                for p in p_order:
                    q0 = p
                    while q0 < M:
                        ws = min(QCAP, M - q0)  # segment [q0, q0+ws)
                        kt = ktp.tile([P, 3, QCAP, g], f16, tag="kt")
                        ktb = kt[:, :, :, :]
                        # fused: (k1, k2, k3n) = (S[p], Rp, Ip)*(R, Sn, D)[q0:]
                        pick(3 * ws * g, 0.53).tensor_tensor(
                            out=_ap(ktb, 0, [[cK, 3], [g, ws], [1, g]], bass),
                            in0=_ap(zzgsb, SL_S * cA + p * g,
                                    [[cA, 3], [0, ws], [1, g]], bass),
                            in1=_ap(zzgsb, SL_R * cA + q0 * g,
                                    [[2 * cA, 3], [g, ws], [1, g]], bass),
                            op=mult)
                        # re = k1 + k3n over q in [q0, q0+ws)
                        emit("re", p, q0 - p, ktb, 0, ktb, 2 * cK, ws)
                        # im = k1 + k2 over q > p
                        sk = g if q0 == p else 0  # skip diagonal col
                        if ws * g - sk > 0:
                            emit("im", p, q0 + sk // g - (p + 1), ktb, sk,
                                 ktb, cK + sk, ws - sk // g)
                        q0 += ws
                for kind in ("re", "im"):
                    assert not st[kind]["tile"], (kind, st[kind]["left"])

    nc.finalize()
    return nc


_CACHED = {}


def _get_nc():
    if "nc" not in _CACHED:
        _CACHED["nc"] = build_bass()
    return _CACHED["nc"]


def kernel(z_re, z_im):
    from concourse.bass_utils import run_bass_kernel_spmd

    z_re = np.ascontiguousarray(np.asarray(z_re, dtype=np.float32))
    z_im = np.ascontiguousarray(np.asarray(z_im, dtype=np.float32))
    assert z_re.shape == (B_FULL, N), z_re.shape

    nc = _get_nc()
    ident = np.eye(P, dtype=np.float16)
    in_maps = []
    for c in range(NC):
        sl = slice(c * B_LOCAL, (c + 1) * B_LOCAL)
        in_maps.append({
            "z_re": np.ascontiguousarray(z_re[sl]),
            "z_im": np.ascontiguousarray(z_im[sl]),
            "ident": ident,
        })
    res = run_bass_kernel_spmd(nc, in_maps, core_ids=list(range(NC)))
    out = np.concatenate([res.results[c]["out"] for c in range(NC)], axis=0)
    return out.astype(np.float32)
